# revision 1
# baseline (speedup 1.0000x reference)
"""EquivariantAttention Trainium2 kernel.

B=8 batches data-parallel over 8 NeuronCores; per core:
  qkv = x @ W_qkv + b_qkv ; dist = cdist(g, g)
  S^T[j,i] = (q_i.k_j)/sqrt(H) * exp(-dist)   (computed transposed: j on partitions)
  U^T = exp(S^T)  (no max-subtraction; values bounded)
  out^T[h,i] = V^T @ U^T, normalized by row-sums l_i (ones-matmul on PE)
  y = out @ W_out + b_out

Matmuls run as float32r (full-rate on PE at moving-dim >= 256); attention
weights U^T and E=exp(-dist) are bf16 (dist itself is f32). d2 is computed
on PE with augmented features [g, |g|^2, 1] . [-2g, 1, |g|^2] packed into
one SBUF tile at partition offsets 0 and 32.

ACT table sets: sqrt and exp live in different sets and a table load costs
~2.7us, so per i-chunk the ACT work is batched [sqrt x16][exp x16][expU x16]
-> 2 loads per chunk. All copies/elementwise ops are pinned to DVE.
"""

import numpy as np

import concourse.bass as bass
from concourse import bacc
import concourse.mybir as mybir
import concourse.tile as tile
from concourse.masks import make_identity
from concourse.tile import add_dep_helper

P = 128
H = 512
SC = 512
HT = H // P  # 4

f32 = mybir.dt.float32
f32r = mybir.dt.float32r
bf16 = mybir.dt.bfloat16
AF = mybir.ActivationFunctionType
OP = mybir.AluOpType
RSQRT_H = 1.0 / float(np.sqrt(H))


def _body(tc, n, x, g, wqkv, bqkv, wout, bout, y):
    nc = tc.nc
    NT = n // P
    NC_ = n // SC
    ITC = SC // P  # i-tiles per chunk (4)

    with (
        nc.allow_low_precision(
            reason="float32r tiles feed FP32r matmuls; storage is fp32-width"
        ),
        tc.tile_pool(name="const", bufs=1) as const,
        tc.tile_pool(name="geo", bufs=1) as geo,
        tc.tile_pool(name="et_pool", bufs=2) as et_pool,
        tc.tile_pool(name="small", bufs=2) as small,
        tc.tile_pool(name="ps_s", bufs=2, space="PSUM") as ps_s,
        tc.tile_pool(name="ps_d", bufs=2, space="PSUM") as ps_d,
        tc.tile_pool(name="ps_o", bufs=3, space="PSUM") as ps_o,
        tc.tile_pool(name="ps_l", bufs=1, space="PSUM") as ps_l,
    ):
        # ---- persistent weights / constants ----
        wout_sb = const.tile([P, HT, H], f32r)
        nc.sync.dma_start(
            wout_sb, wout.rearrange("(kt p) m -> p kt m", p=P).bitcast(f32r)
        )
        bqk_sb = const.tile([P, 8], f32)  # cols 0-3: b_q m-tiles, 4-7: b_k
        nc.sync.dma_start(bqk_sb, bqkv[0 : 2 * H].rearrange("(mt p) -> p mt", p=P))
        # pre-scale q biases so the ACT Identity copyback computes (ps + b)*s
        # as ps*s + b*s with bias AP = b*s
        nc.vector.tensor_scalar_mul(bqk_sb[:, 0:4], bqk_sb[:, 0:4], RSQRT_H)
        bo_bc = const.tile([P, H], f32)
        nc.gpsimd.dma_start(bo_bc, bout.partition_broadcast(P))
        ones_bf = const.tile([P, 1], bf16)
        nc.vector.memset(ones_bf, 1.0)
        ones_row = const.tile([1, P], f32r)
        nc.vector.memset(ones_row.bitcast(f32), 1.0)

        # augmented geometry, transposed: d2[j,i] = sum_k h_k[j] * g_k[i]
        hT8 = geo.tile([8, n], f32r)
        gT8 = geo.tile([8, n], f32r)

        # ---- q/k/v projection ----
        with tc.tile_pool(name="qkv", bufs=1) as qkv:
            qT = qkv.tile([P, HT, n], f32r)  # q^T / sqrt(H), [h, i]
            kT = qkv.tile([P, HT, n], f32r)  # k^T, [h, j]
            v_bf = qkv.tile([P, NT, H], bf16)  # v natural, [j, h]

            with tc.tile_pool(name="xt_pool", bufs=1) as xt_pool:
                xT = xt_pool.tile([P, HT, n], f32r)
                ident = xt_pool.tile([P, P], f32)
                make_identity(nc, ident)

                with tc.tile_pool(name="wstage", bufs=1) as wstage:
                    # weight DMA first so it overlaps the x transposes
                    wqkv_sb = wstage.tile([P, HT, 3 * H], f32r)
                    nc.gpsimd.dma_start(
                        wqkv_sb,
                        wqkv.rearrange("(kt p) m -> p kt m", p=P).bitcast(f32r),
                    )
                    bv_bc = wstage.tile([P, H], f32)
                    nc.gpsimd.dma_start(
                        bv_bc, bqkv[2 * H : 3 * H].partition_broadcast(P)
                    )
                    g_sb = wstage.tile([P, NT, 3], f32)
                    nc.sync.dma_start(g_sb, g.rearrange("(nt p) c -> p nt c", p=P))
                    g2 = wstage.tile([P, NT, 3], f32)
                    nc.vector.tensor_mul(g2, g_sb, g_sb)
                    sq = wstage.tile([P, NT, 1], f32)
                    nc.vector.reduce_sum(sq, g2, axis=mybir.AxisListType.X)
                    Ag = wstage.tile([P, NT, 8], f32)  # [g, |g|^2, 1, 0..]
                    Ah = wstage.tile([P, NT, 8], f32)  # [-2g, 1, |g|^2, 0..]
                    nc.vector.memset(Ag, 0.0)
                    nc.vector.memset(Ah, 0.0)
                    nc.vector.tensor_copy(Ag[:, :, 0:3], g_sb)
                    nc.vector.tensor_copy(Ag[:, :, 3:4], sq)
                    nc.vector.memset(Ag[:, :, 4:5], 1.0)
                    nc.vector.tensor_scalar_mul(Ah[:, :, 0:3], g_sb, -2.0)
                    nc.vector.memset(Ah[:, :, 3:4], 1.0)
                    nc.vector.tensor_copy(Ah[:, :, 4:5], sq)
                    for nt in range(NT):
                        pt = ps_s.tile([P, SC], f32, tag="pss")
                        nc.tensor.transpose(pt[:8, :P], Ah[:, nt, :], ident)
                        nc.scalar.copy(hT8[:, nt * P : (nt + 1) * P], pt[:8, :P])
                        pt2 = ps_d.tile([P, SC], f32, tag="psd")
                        nc.tensor.transpose(pt2[:8, :P], Ag[:, nt, :], ident)
                        nc.scalar.copy(gT8[:, nt * P : (nt + 1) * P], pt2[:8, :P])
                    NH = max(1, NT // 4)
                    x_r = x.rearrange("(nt p) h -> p nt h", p=P)
                    with tc.tile_pool(name="xsb_pool", bufs=2) as xsb_pool:
                        for qi, hh in enumerate(range(0, NT, NH)):
                            x_sb = xsb_pool.tile([P, NH, H], f32, tag="x_sb")
                            eng = (nc.sync, nc.scalar)[qi % 2]
                            eng.dma_start(x_sb, x_r[:, hh : hh + NH, :])
                            for nt in range(NH):
                                for ht in range(HT):
                                    tp_pool, tp_tag = (
                                        (ps_o, "pso"),
                                        (ps_s, "pss"),
                                        (ps_d, "psd"),
                                    )[(nt * HT + ht) % 3]
                                    pt = tp_pool.tile([P, SC], f32, tag=tp_tag)
                                    nc.tensor.transpose(
                                        pt[:, :P],
                                        x_sb[:, nt, ht * P : (ht + 1) * P],
                                        ident,
                                    )
                                    dst_ap = xT[
                                        :, ht, (hh + nt) * P : (hh + nt + 1) * P
                                    ]
                                    if ht % 2 == 0:
                                        nc.scalar.copy(dst_ap, pt[:, :P])
                                    else:
                                        nc.vector.tensor_copy(dst_ap, pt[:, :P])

                    # q^T and k^T tiles; m-tile mt in 0..8 (0-3 q, 4-7 k)
                    for mt in range(8):
                        dst = qT if mt < 4 else kT
                        mi = mt % 4
                        scale = RSQRT_H if mt < 4 else 1.0
                        for c in range(NC_):
                            ps = ps_s.tile([P, SC], f32, tag="pss")
                            for kc in range(HT):
                                nc.tensor.matmul(
                                    ps,
                                    lhsT=wqkv_sb[:, kc, mt * P : (mt + 1) * P],
                                    rhs=xT[:, kc, c * SC : (c + 1) * SC],
                                    start=(kc == 0),
                                    stop=(kc == HT - 1),
                                )
                            # out = ps*scale + (b*scale) on ACT (idle in prologue)
                            nc.scalar.activation(
                                dst[:, mi, c * SC : (c + 1) * SC],
                                ps,
                                AF.Identity,
                                bias=bqk_sb[:, mt : mt + 1],
                                scale=scale,
                            )

                    # v natural: [j, h] (bf16), bias added
                    for nt in range(NT):
                        ps = ps_s.tile([P, SC], f32, tag="pss")
                        for kc in range(HT):
                            nc.tensor.matmul(
                                ps,
                                lhsT=xT[:, kc, nt * P : (nt + 1) * P],
                                rhs=wqkv_sb[:, kc, 2 * H : 3 * H],
                                start=(kc == 0),
                                stop=(kc == HT - 1),
                            )
                        nc.vector.tensor_add(v_bf[:, nt, :], ps, bv_bc)

            # ---- attention, transposed, software-pipelined over i-chunks ----
            with (
                tc.tile_pool(name="dist_pool", bufs=1) as dist_pool,
                tc.tile_pool(name="e_pool", bufs=1) as e_pool,
                tc.tile_pool(name="ut_pool", bufs=2) as ut_pool,
                tc.tile_pool(name="ot_pool", bufs=1) as ot_pool,
            ):
                y_r = y.rearrange("(nt p) h -> p nt h", p=P)
                state = {"prev": None}

                def chain(a):
                    # keep ACT in emission order so sqrt/exp table-set
                    # switches happen once per block, not per op
                    if state["prev"] is not None:
                        add_dep_helper(
                            a.ins,
                            state["prev"].ins,
                            sync=False,
                            reason="ACT table-set batching",
                        )
                    state["prev"] = a
                    return a

                dists = {}
                Es = {}

                def emit_sqrt(ic, jts=None):
                    # d2 on PE + clamp + sqrt block for chunk ic
                    isl = slice(ic * SC, (ic + 1) * SC)
                    if ic in dists:
                        dist = dists[ic]
                    else:
                        dist = dist_pool.tile([P, NT, SC], f32, tag="dist")
                        dists[ic] = dist
                    jl = list(jts) if jts is not None else list(range(NT))
                    for jt in jl:
                        psd = ps_d.tile([P, SC], f32, tag="psd")
                        nc.tensor.matmul(
                            psd,
                            lhsT=hT8[:, jt * P : (jt + 1) * P],
                            rhs=gT8[:, isl],
                            start=True,
                            stop=True,
                        )
                        # clamp writes PSUM->SBUF so the psd bank frees here,
                        # and sqrt can run in-place on SBUF pairs (1024-wide)
                        nc.vector.tensor_scalar_max(dist[:, jt, :], psd, 0.0)
                        if jt % 2 == 1 or jt == jl[-1]:
                            j0 = jt - (jt % 2)
                            chain(
                                nc.scalar.activation(
                                    dist[:, j0 : jt + 1, :],
                                    dist[:, j0 : jt + 1, :],
                                    AF.Sqrt,
                                )
                            )

                def emit_exp(ic):
                    # E = exp(-dist) block (pairs of j-tiles per ACT op)
                    dist = dists.pop(ic)
                    E = e_pool.tile([P, NT, SC], bf16, tag="E")
                    step = 4 if NT % 4 == 0 else 1
                    for jp in range(0, NT, step):
                        chain(
                            nc.scalar.activation(
                                E[:, jp : jp + step, :],
                                dist[:, jp : jp + step, :],
                                AF.Exp,
                                scale=-1.0,
                            )
                        )
                    Es[ic] = E

                emit_sqrt(0)
                emit_exp(0)
                for ic in range(NC_):
                    isl = slice(ic * SC, (ic + 1) * SC)
                    E = Es.pop(ic)
                    UT = ut_pool.tile([P, NT, SC], bf16, tag="UT")
                    psl = ps_l.tile([1, SC], f32, tag="psl")
                    for jt in range(NT):
                        jsl = slice(jt * P, (jt + 1) * P)
                        pss = ps_s.tile([P, SC], f32, tag="pss")
                        for kc in range(HT):
                            nc.tensor.matmul(
                                pss,
                                lhsT=kT[:, kc, jsl],
                                rhs=qT[:, kc, isl],
                                start=(kc == 0),
                                stop=(kc == HT - 1),
                            )
                        if jt % 2 == 0:
                            et2 = et_pool.tile([P, 2, SC], f32, tag="et")
                        nc.vector.tensor_mul(et2[:, jt % 2, :], pss, E[:, jt, :])
                        if jt % 2 == 1:
                            # exp + row-sum matmul over the pair of j-tiles
                            chain(
                                nc.scalar.activation(
                                    UT[:, jt - 1 : jt + 1, :], et2, AF.Exp
                                )
                            )
                            nc.tensor.matmul(
                                psl,
                                lhsT=ones_bf,
                                rhs=UT[:, jt - 1, :],
                                start=(jt == 1),
                                stop=False,
                            )
                            nc.tensor.matmul(
                                psl,
                                lhsT=ones_bf,
                                rhs=UT[:, jt, :],
                                start=False,
                                stop=(jt == NT - 1),
                            )
                    # pipeline: next chunk's E phase (early start for ACT)
                    if ic + 1 < NC_:
                        emit_sqrt(ic + 1)
                        emit_exp(ic + 1)
                    linv_row = et_pool.tile([1, SC], f32r, tag="et")
                    nc.vector.reciprocal(linv_row, psl)
                    # broadcast 1/l across partitions via K=1 matmul
                    psb = ps_d.tile([P, SC], f32, tag="psd")
                    nc.tensor.matmul(
                        psb, lhsT=ones_row, rhs=linv_row, start=True, stop=True
                    )
                    lbc = et_pool.tile([P, SC], f32, tag="et")
                    nc.vector.tensor_copy(lbc, psb)
                    # out^T[h, i] = V^T @ U^T, scaled by 1/l_i.
                    # Next chunk's d2+clamp+sqrt is interleaved per attnV group
                    # so DVE clamps never head-of-line block the outT multiplies
                    # and the d2 matmuls fill PE slot-wait gaps.
                    outT = ot_pool.tile([P, HT, SC], f32r, tag="outT")
                    for ht in range(HT):
                        pso = ps_o.tile([P, SC], f32, tag="pso")
                        for jt in range(NT):
                            nc.tensor.matmul(
                                pso,
                                lhsT=v_bf[:, jt, ht * P : (ht + 1) * P],
                                rhs=UT[:, jt, :],
                                start=(jt == 0),
                                stop=(jt == NT - 1),
                            )
                        nc.vector.tensor_mul(outT[:, ht, :], pso, lbc)
                    # final projection for this chunk's i-tiles; bias added
                    # in-place in PSUM, DMA reads PSUM directly
                    for it4 in range(ITC):
                        psy = ps_o.tile([P, SC], f32, tag="pso")
                        for ht in range(HT):
                            nc.tensor.matmul(
                                psy,
                                lhsT=outT[:, ht, it4 * P : (it4 + 1) * P],
                                rhs=wout_sb[:, ht, :],
                                start=(ht == 0),
                                stop=(ht == HT - 1),
                            )
                        ysb = small.tile([P, H], f32, tag="ysb")
                        nc.vector.tensor_add(ysb, psy, bo_bc)
                        nc.sync.dma_start(y_r[:, ic * ITC + it4, :], ysb)


def build_bass(n: int = 2048) -> bass.Bass:
    nc = bacc.Bacc(None, target_bir_lowering=False)
    x = nc.dram_tensor("x", [n, H], f32, kind="ExternalInput")[:, :]
    g = nc.dram_tensor("g", [n, 3], f32, kind="ExternalInput")[:, :]
    wqkv = nc.dram_tensor("w_qkv", [H, 3 * H], f32, kind="ExternalInput")[:, :]
    bqkv = nc.dram_tensor("b_qkv", [3 * H], f32, kind="ExternalInput")[:]
    wout = nc.dram_tensor("w_out", [H, H], f32, kind="ExternalInput")[:, :]
    bout = nc.dram_tensor("b_out", [H], f32, kind="ExternalInput")[:]
    y = nc.dram_tensor("y", [n, H], f32, kind="ExternalOutput")[:, :]
    with tile.TileContext(nc) as tc:
        _body(tc, n, x, g, wqkv, bqkv, wout, bout, y)
    nc.finalize()
    return nc


_CACHED = {}


def _get_nc(n: int = 2048) -> bass.Bass:
    if n not in _CACHED:
        _CACHED[n] = build_bass(n)
    return _CACHED[n]


def kernel(**inputs) -> np.ndarray:
    from concourse.bass_utils import run_bass_kernel_spmd

    x = np.ascontiguousarray(inputs["x"], dtype=np.float32)
    g = np.ascontiguousarray(inputs["geometric_features"], dtype=np.float32)
    wqkv = np.ascontiguousarray(inputs["W_qkv"], dtype=np.float32)
    bqkv = np.ascontiguousarray(inputs["b_qkv"], dtype=np.float32)
    wout = np.ascontiguousarray(inputs["W_out"], dtype=np.float32)
    bout = np.ascontiguousarray(inputs["b_out"], dtype=np.float32)

    B, n, _ = x.shape
    nc = _get_nc(n)
    core_ids = list(range(B))
    in_maps = [
        {
            "x": np.ascontiguousarray(x[b]),
            "g": np.ascontiguousarray(g[b]),
            "w_qkv": wqkv,
            "b_qkv": bqkv,
            "w_out": wout,
            "b_out": bout,
        }
        for b in range(B)
    ]
    res = run_bass_kernel_spmd(nc, in_maps, core_ids)
    return np.stack([res.results[b]["y"] for b in range(B)]).astype(np.float32)



# revision 6
# speedup vs baseline: 1.4887x; 1.4887x over previous
"""EquivariantAttention Trainium2 kernel, v2.

B=8 batches data-parallel over 8 NeuronCores; per core (n=2048, H=512):

  qkv = x @ W_qkv + b ; only q,k,v'' kept where v'' = x @ (W_v@W_out) + b''
  (W_out folded into V so the final projection disappears; b'' = b_v@W_out+b_out
   and the +b_out part rides the softmax identity sum_j p_ij = 1).

  E = exp(-sqrt(d2 + eps)) precomputed f16 for ALL i-chunks in the prologue
  (one Sqrt table block + one Exp table block = 2 ACT table loads total);
  d2 computed on PE from augmented geometry, sqrt reads PSUM directly
  (eps bias replaces the DVE clamp).

  scores S^T[j,i] = k8^T q8 via fp8e4 DoubleRow matmuls (K=256/instr,
  0.5 cycles/row); q split hi+lo fp8 for precision, k single fp8.
  U = exp(S^T*E/sqrt(H)) as f16, W = U-1 quantized e4m3 (near-1 precision),
  attnV natural layout: out[i,h] = colsum(v'') + W8 @ (v_hi8 + v_lo8) via
  DoubleRow; rowsums l = 2048 + W8 @ ones via ap=1 DR matmuls; y = out/l.

Engine split: PE matmuls; ACT sqrt/exp/ynorm; DVE psum copybacks + s*E;
Pool (SBUF-only operands for HW safety) dtype converts + W=U-1 subtract.
"""

import numpy as np

import concourse.bass as bass
from concourse import bacc
import concourse.mybir as mybir
import concourse.tile as tile
from concourse.masks import make_identity
from concourse.tile import add_dep_helper

P = 128
H = 512
SC = 512
HT = H // P  # 4

f32 = mybir.dt.float32
f32r = mybir.dt.float32r
bf16 = mybir.dt.bfloat16
f16 = mybir.dt.float16
fp8 = mybir.dt.float8e4
AF = mybir.ActivationFunctionType
OP = mybir.AluOpType
DR = mybir.MatmulPerfMode.DoubleRow
RSQRT_H = 1.0 / float(np.sqrt(H))
D2_EPS = 1e-4
EPS_OVERRIDE = [None]


def _body(tc, n, x, g, wqkv, bqkv, wout, bout, y, dbg=None):
    nc = tc.nc
    NT = n // P           # j-tiles (16)
    NC_ = n // SC         # i-chunks (4)
    ITC = SC // P         # i-tiles per chunk (4)
    NH = 2                # x-DMA chunk, in units of nt
    NJJ = NT // 2         # j pair-tiles (8)

    state = {"prev": None}

    def chain(a):
        # keep ACT in emission order so table-set switches stay batched
        if state["prev"] is not None:
            add_dep_helper(a.ins, state["prev"].ins, sync=False,
                           reason="ACT table-set batching")
        state["prev"] = a
        return a

    with (
        nc.allow_low_precision(
            reason="fp8/bf16 attention; fp32r transposes and d2"
        ),
        tc.tile_pool(name="const", bufs=1) as const,
        tc.tile_pool(name="attn", bufs=1) as attn,
        tc.tile_pool(name="etp", bufs=2) as etp,
        tc.tile_pool(name="u16p", bufs=2) as u16p,
        tc.tile_pool(name="w8p", bufs=2) as w8p,
        tc.tile_pool(name="yp", bufs=2) as yp,
        tc.tile_pool(name="lp", bufs=2) as lp,
    ):
        # ---------------- constants ----------------
        ident = const.tile([P, P], f32)
        make_identity(nc, ident)
        eps_ap = const.tile([P, 1], f32)
        nc.vector.memset(eps_ap, EPS_OVERRIDE[0] or D2_EPS)
        bqk_sb = const.tile([P, 8], f32)  # cols 0-3 b_q m-tiles, 4-7 b_k
        nc.sync.dma_start(bqk_sb, bqkv[0 : 2 * H].rearrange("(mt p) -> p mt", p=P))
        ones8 = const.tile([P, 2, 1], fp8)
        nc.vector.memset(ones8, 1.0)
        ones_row = const.tile([1, P], f32r)
        nc.vector.memset(ones_row.bitcast(f32), 1.0)
        cN = const.tile([1, ITC], f32r)
        nc.vector.memset(cN.bitcast(f32), float(n))
        colsum_row = const.tile([1, H], f32r)  # written in prologue

        # ---------------- persistent attention tiles ----------------
        qhT = attn.tile([P, HT, n], fp8)   # q hi, [h, i] natural scale
        qlT = attn.tile([P, HT, n], fp8)   # q lo residual
        kT = attn.tile([P, HT, n], fp8)    # k, [h, j]
        Es = attn.tile([P, NC_, NT, SC], f16)  # exp(-dist), [j, i]
        v8h = attn.tile([P, NT, H], fp8)   # v'' hi, [j, h]
        v8l = attn.tile([P, NT, H], fp8)   # v'' lo

        # ================= prologue =================
        with (
            tc.tile_pool(name="geo", bufs=1) as geo,
            tc.tile_pool(name="wsb", bufs=1) as wsb,
            tc.tile_pool(name="xsb", bufs=3) as xsb,
            tc.tile_pool(name="xtc", bufs=2) as xtc,
            tc.tile_pool(name="vt", bufs=1) as vt,
            tc.tile_pool(name="ps_q", bufs=3, space="PSUM") as ps_q,  # 3x1
            tc.tile_pool(name="ps_d", bufs=2, space="PSUM") as ps_d,  # 2x2
            tc.tile_pool(name="ps_one", bufs=1, space="PSUM") as ps_one,  # 1
        ):
            # -- DMA priority order (single DMA engine serializes): g, x0,
            # Wv+Wout pieces (W' needed early), then x chunks alternating
            # with remaining W_qkv pieces; weights stage f32->bf16 on Pool.
            WPC = 128
            g_sb = wsb.tile([P, NT, 3], f32)
            nc.sync.dma_start(g_sb, g.rearrange("(nt p) c -> p nt c", p=P))
            x_r = x.rearrange("(nt p) h -> p nt h", p=P)
            wq_r = wqkv.rearrange("(kt p) m -> p kt m", p=P)
            wo_r = wout.rearrange("(kt p) m -> p kt m", p=P)
            wqkv_bf = wsb.tile([P, HT, 3 * H], bf16)
            wout_bf = wsb.tile([P, HT, H], bf16)

            def stage_w(dst_bf, src_ap):
                # DMA issue on the (otherwise idle) SP queue so the Pool
                # converts don't serialize the staging DMAs behind them
                ws = xsb.tile([P, HT, WPC], f32, tag="ws")
                nc.sync.dma_start(ws, src_ap)
                nc.gpsimd.tensor_copy(dst_bf, ws)

            x_stage = []

            def dma_x(i):
                xs = xsb.tile([P, NH, H], f32, tag="xs")
                nc.sync.dma_start(xs, x_r[:, i * NH : (i + 1) * NH, :])
                x_stage.append(xs)

            for i in range(min(4, NT // NH)):
                dma_x(i)
            for pc in range(8, 12):  # Wv
                stage_w(wqkv_bf[:, :, pc * WPC : (pc + 1) * WPC],
                        wq_r[:, :, pc * WPC : (pc + 1) * WPC])
            for pc in range(4):  # Wout
                stage_w(wout_bf[:, :, pc * WPC : (pc + 1) * WPC],
                        wo_r[:, :, pc * WPC : (pc + 1) * WPC])
            nxt = [0]

            def stage_wq4():
                for _ in range(4):
                    pc = nxt[0]
                    if pc < 8:
                        stage_w(wqkv_bf[:, :, pc * WPC : (pc + 1) * WPC],
                                wq_r[:, :, pc * WPC : (pc + 1) * WPC])
                        nxt[0] = pc + 1

            for i in range(4, NT // NH):
                if i in (4, 5):
                    stage_wq4()
                dma_x(i)
            while nxt[0] < 8:
                stage_wq4()
            bv_sb = wsb.tile([P, HT], f32)
            nc.gpsimd.dma_start(
                bv_sb, bqkv[2 * H : 3 * H].rearrange("(kt p) -> p kt", p=P)
            )
            bpp_row = wsb.tile([1, H], f32r)
            nc.gpsimd.dma_start(
                bpp_row,
                bout.rearrange("(one h) -> one h", one=1).bitcast(f32r))
            bv_bf = wsb.tile([P, HT], bf16)
            nc.gpsimd.tensor_copy(bv_bf, bv_sb)

            # -- augmented geometry in EXACT f16: d2 = |g16_i - g16_j|^2
            # via one K=8 f16 matmul; f16xf16 products are exact in the f32
            # accumulator so d2 >= ~-1e-5 and sqrt(d2+eps) never NaNs on HW
            # (f32r-rounded operands gave d2 errors of +-6e-3 -> NaNs).
            # lhsT rows: [-2g(3), sq_hi, sq_lo, 1, 1, 0]
            # rhs  rows: [ g(3),  1,    1,     sq_hi, sq_lo, 0]
            ident16 = wsb.tile([P, P], f16)
            nc.vector.tensor_copy(ident16, ident)
            ident_bf = wsb.tile([P, P], bf16)
            nc.vector.tensor_copy(ident_bf, ident)
            hT8 = geo.tile([8, n], f16)
            gT8 = geo.tile([8, n], f16)
            g16 = wsb.tile([P, NT, 3], f16)
            nc.vector.tensor_copy(g16, g_sb)
            g2 = wsb.tile([P, NT, 3], f32)
            nc.vector.tensor_mul(g2, g16, g16)
            sq = wsb.tile([P, NT, 1], f32)
            nc.vector.reduce_sum(sq, g2, axis=mybir.AxisListType.X)
            sqh = wsb.tile([P, NT, 1], f16)
            nc.vector.tensor_copy(sqh, sq)
            sql = wsb.tile([P, NT, 1], f16)
            nc.vector.tensor_sub(sql, sq, sqh)
            Ag = wsb.tile([P, NT, 8], f16)
            Ah = wsb.tile([P, NT, 8], f16)
            nc.vector.memset(Ag, 0.0)
            nc.vector.memset(Ah, 0.0)
            nc.vector.tensor_copy(Ag[:, :, 0:3], g16)
            nc.vector.memset(Ag[:, :, 3:5], 1.0)
            nc.vector.tensor_copy(Ag[:, :, 5:6], sqh)
            nc.vector.tensor_copy(Ag[:, :, 6:7], sql)
            nc.vector.tensor_scalar_mul(Ah[:, :, 0:3], g16, -2.0)
            nc.vector.tensor_copy(Ah[:, :, 3:4], sqh)
            nc.vector.tensor_copy(Ah[:, :, 4:5], sql)
            nc.vector.memset(Ah[:, :, 5:7], 1.0)
            for q4 in range(NT // 4):
                for srcA, dstT in ((Ah, hT8), (Ag, gT8)):
                    pq32 = ps_q.tile([P, 4, P], f32, tag="psq")
                    pq = pq32.bitcast(f16)  # [P, 4, 2P]
                    for j in range(4):
                        nt = q4 * 4 + j
                        nc.tensor.transpose(
                            pq[:8, j, 0:P], srcA[:, nt, :], ident16
                        )
                    nc.vector.tensor_copy(
                        dstT[:, q4 * 4 * P : (q4 + 1) * 4 * P],
                        pq[:8, :, 0:P],
                    )

            def emit_d2_chunk(ic):
                # d2 for i-chunk ic: pair-tiles -> psd -> ACT sqrt -> Es f16
                isl = slice(ic * SC, (ic + 1) * SC)
                for jj in range(NJJ):
                    psd = ps_d.tile([P, 2, SC], f32, tag="psd")
                    for s2 in range(2):
                        jt = jj * 2 + s2
                        nc.tensor.matmul(
                            psd[:, s2, :],
                            lhsT=hT8[:, jt * P : (jt + 1) * P],
                            rhs=gT8[:, isl],
                            start=True,
                            stop=True,
                        )
                    chain(
                        nc.scalar.activation(
                            Es[:, ic, jj * 2 : jj * 2 + 2, :],
                            psd,
                            AF.Sqrt,
                            bias=eps_ap,
                        )
                    )

            # -- per n-chunk: x transposes -> xTc (bf16), d2, v'', q/k proj
            # (proj delayed one chunk so its W_qkv pieces have arrived)
            Wp = wsb.tile([P, HT, H], bf16)  # W' = Wv @ Wout, k on partitions
            WvT = wsb.tile([P, HT, H], bf16)  # Wv^T: [h', k]
            pcs = ps_one.tile([1, H], f32, tag="pcs")
            xtc_tiles = {}

            def emit_wprep():
                for kt in range(HT):
                    pq32 = ps_q.tile([P, HT, P], f32, tag="psq")
                    pq = pq32.bitcast(bf16)  # [P, HT, 2*P]
                    for ht in range(HT):
                        nc.tensor.transpose(
                            pq[:, ht, 0:P],
                            wqkv_bf[:, kt,
                                    2 * H + ht * P : 2 * H + (ht + 1) * P],
                            ident_bf,
                        )
                    nc.vector.tensor_copy(
                        WvT[:, :, kt * P : (kt + 1) * P], pq[:, :, 0:P]
                    )
                for kt in range(HT):
                    psp = ps_q.tile([P, 4, P], f32, tag="psq")
                    for hp in range(HT):
                        nc.tensor.matmul(
                            psp,
                            lhsT=WvT[:, hp, kt * P : (kt + 1) * P],
                            rhs=wout_bf[:, hp, :],
                            start=(hp == 0),
                            stop=(hp == HT - 1),
                        )
                    nc.vector.tensor_copy(Wp[:, kt, :], psp)
                # b'' = b_v @ W_out + b_out, broadcast to [P, H]
                psb4 = ps_q.tile([P, 4, P], f32, tag="psq")
                psb = psb4[0:1, :, :]
                for kt in range(HT):
                    nc.tensor.matmul(
                        psb,
                        lhsT=bv_bf[:, kt : kt + 1],
                        rhs=wout_bf[:, kt, :],
                        start=(kt == 0),
                        stop=(kt == HT - 1),
                    )
                nc.vector.tensor_add(bpp_row, psb, bpp_row)

            def emit_vchunk(cv):
                xTc = xtc_tiles[cv]
                for ntl in range(ITC):
                    nt = cv * ITC + ntl
                    psp = ps_q.tile([P, 4, P], f32, tag="psq")
                    nc.tensor.matmul(
                        psp,
                        lhsT=ones_row,
                        rhs=bpp_row,
                        start=True,
                        stop=False,
                        skip_group_check=True,
                    )
                    for kc in range(HT):
                        nc.tensor.matmul(
                            psp,
                            lhsT=xTc[:, kc, ntl * P : (ntl + 1) * P],
                            rhs=Wp[:, kc, :],
                            start=False,
                            stop=(kc == HT - 1),
                            skip_group_check=True,
                        )
                    vtmp = vt.tile([P, H], f32, tag="vt")
                    nc.vector.tensor_copy(vtmp, psp)
                    nc.gpsimd.tensor_copy(v8h[:, nt, :], vtmp)
                    nc.vector.tensor_sub(v8l[:, nt, :], vtmp, v8h[:, nt, :])

            def emit_qkproj(c):
                csl = slice(c * SC, (c + 1) * SC)
                xTc = xtc_tiles.pop(c)
                for mt in [0, 1, 2, 3, 4, 5, 6, 7]:
                    mi = mt % 4
                    psp = ps_q.tile([P, 4, P], f32, tag="psq")
                    for kc in range(HT):
                        nc.tensor.matmul(
                            psp,
                            lhsT=wqkv_bf[:, kc, mt * P : (mt + 1) * P],
                            rhs=xTc[:, kc, :],
                            start=(kc == 0),
                            stop=(kc == HT - 1),
                        )
                    if mt >= 4:
                        nc.vector.tensor_scalar_add(
                            kT[:, mi, csl], psp, bqk_sb[:, 4 + mi : 5 + mi]
                        )
                    else:
                        nc.vector.tensor_scalar_add(
                            qhT[:, mi, csl], psp, bqk_sb[:, mi : mi + 1]
                        )
                        nc.vector.scalar_tensor_tensor(
                            qlT[:, mi, csl],
                            psp,
                            bqk_sb[:, mi : mi + 1],
                            qhT[:, mi, csl],
                            OP.add,
                            OP.subtract,
                        )

            for c in range(NC_):
                xTc = xtc.tile([P, HT, SC], bf16, tag="xtc")
                xtc_tiles[c] = xTc
                for ht in range(HT):
                    pq = ps_q.tile([P, 4, P], f32, tag="psq")
                    for j in range(4):
                        xs = x_stage[2 * c + j // NH]
                        nc.tensor.transpose(
                            pq[:, j, :],
                            xs[:, j % NH, ht * P : (ht + 1) * P],
                            ident,
                        )
                    nc.vector.tensor_copy(xTc[:, ht, :], pq)
                emit_d2_chunk(c)
                if c == min(1, NC_ - 1):
                    emit_wprep()
                for cv in ([c - 1] if c >= 1 else ([] if NC_ > 1 else [0])):
                    emit_vchunk(cv)
                if c >= 1:
                    emit_qkproj(c - 1)
            if NC_ > 1:
                emit_vchunk(NC_ - 1)
            emit_qkproj(NC_ - 1)
            if dbg is not None:
                nc.sync.dma_start(dbg["ht8"], hT8)
                nc.sync.dma_start(dbg["gt8"], gT8)
            # column sums of v'' from the fp8 hi/lo pair (plain fp8 matmuls;
            # DoubleRow with a 1-partition output breaks walrus codegen)
            ones81 = const.tile([P, 1], fp8)
            nc.vector.memset(ones81, 1.0)
            for v8 in (v8h, v8l):
                for jt in range(NT):
                    nc.tensor.matmul(
                        pcs,
                        lhsT=ones81,
                        rhs=v8[:, jt, :],
                        start=(v8 is v8h and jt == 0),
                        stop=(v8 is v8l and jt == NT - 1),
                    )
            nc.vector.tensor_copy(colsum_row, pcs)

            # E = exp(-dist), in place on Es (f16), 4 j-tiles per op;
            # then pin the diagonal back to exactly 1 (E<=1 everywhere, and
            # sqrt(d2+eps) biased the self-distance) via max with identity
            for ic in range(NC_):
                for jq in range(NT // 4):
                    chain(
                        nc.scalar.activation(
                            Es[:, ic, jq * 4 : (jq + 1) * 4, :],
                            Es[:, ic, jq * 4 : (jq + 1) * 4, :],
                            AF.Exp,
                            scale=-1.0,
                        )
                    )
                for it in range(ITC):
                    jd = ic * ITC + it
                    dsl = Es[:, ic, jd, it * P : (it + 1) * P]
                    nc.vector.tensor_max(dsl, dsl, ident16)

        # ================= main attention loop =================
        # open order matters: ps_o/ps_l land on the prologue's psq banks
        # (drained late by the last copybacks), ps_s lands on the d2 banks
        # (drained early) so scores c0 isn't gated on the prologue tail
        main_po = tc.tile_pool(name="ps_o", bufs=2, space="PSUM")
        ps_o = main_po.__enter__()
        main_pl = tc.tile_pool(name="ps_l", bufs=1, space="PSUM")
        ps_l = main_pl.__enter__()
        main_ps = tc.tile_pool(name="ps_s", bufs=2, space="PSUM")
        ps_s = main_ps.__enter__()
        y_r = y.rearrange("(nt p) h -> p nt h", p=P)

        def emit_scores(ic, jjs=None):
            isl = slice(ic * SC, (ic + 1) * SC)
            pss_list = []
            for jj in (jjs if jjs is not None else range(NJJ)):
                pss = ps_s.tile([P, 2, SC], f32, tag="pss")
                for s2 in range(2):
                    jt = jj * 2 + s2
                    jsl = slice(jt * P, (jt + 1) * P)
                    first = True
                    for qT in (qhT, qlT):
                        for kcp in range(HT // 2):
                            nc.tensor.matmul(
                                pss[:, s2, :],
                                lhsT=kT[:, 2 * kcp : 2 * kcp + 2, jsl],
                                rhs=qT[:, 2 * kcp : 2 * kcp + 2, isl],
                                start=first,
                                stop=(qT is qlT) and kcp == HT // 2 - 1,
                                perf_mode=DR,
                            )
                            first = False
                pss_list.append(pss)
            return pss_list

        def emit_elementwise(ic, pss_list, W8, jjs):
            # s*E on DVE, U=exp on ACT, W8=U-1 on Pool
            for i, jj in enumerate(jjs):
                et = etp.tile([P, 2, SC], f16, tag="et")
                nc.vector.tensor_mul(
                    et, pss_list[i], Es[:, ic, jj * 2 : jj * 2 + 2, :]
                )
                u16 = u16p.tile([P, 2, SC], f16, tag="u16")
                chain(nc.scalar.activation(u16, et, AF.Exp, scale=RSQRT_H))
                nc.gpsimd.tensor_scalar(
                    W8[:, jj * 2 : jj * 2 + 2, :], u16, 1.0, 400.0,
                    OP.subtract, OP.min,
                )

        def emit_attnv(ic, W8):
            # rowsums l = n + sum_j W8 (ap=1 DR matmuls into psl columns)
            psl = ps_l.tile([P, ITC], f32, tag="psl")
            for jj in range(NJJ):
                for it in range(ITC):
                    # start only on the very first matmul: the PSUM zero
                    # region is the whole 2KB bank, so a second start would
                    # wipe the other columns' partial sums
                    nc.tensor.matmul(
                        psl[:, it : it + 1],
                        lhsT=W8[:, jj * 2 : jj * 2 + 2, it * P : (it + 1) * P],
                        rhs=ones8,
                        start=(jj == 0 and it == 0),
                        stop=False,
                        perf_mode=DR,
                        skip_group_check=True,
                    )
            nc.tensor.matmul(
                psl,
                lhsT=ones_row,
                rhs=cN,
                start=False,
                stop=True,
                skip_group_check=True,
            )
            linv = lp.tile([P, ITC], f32, tag="linv")
            nc.vector.reciprocal(linv, psl)
            # out[i, h] = colsum + W8 @ (v_hi + v_lo), then y = out * (1/l)
            for it in range(ITC):
                pso = ps_o.tile([P, SC], f32, tag="pso")
                nc.tensor.matmul(
                    pso,
                    lhsT=ones_row,
                    rhs=colsum_row,
                    start=True,
                    stop=False,
                    skip_group_check=True,
                )
                itsl = slice(it * P, (it + 1) * P)
                for v8 in (v8h, v8l):
                    for jj in range(NJJ):
                        nc.tensor.matmul(
                            pso,
                            lhsT=W8[:, jj * 2 : jj * 2 + 2, itsl],
                            rhs=v8[:, jj * 2 : jj * 2 + 2, :],
                            start=False,
                            stop=(v8 is v8l) and jj == NJJ - 1,
                            perf_mode=DR,
                            skip_group_check=True,
                        )
                y_t = yp.tile([P, H], f32, tag="y")
                chain(
                    nc.scalar.activation(
                        y_t, pso, AF.Identity, scale=linv[:, it : it + 1]
                    )
                )
                nc.sync.dma_start(y_r[:, ic * ITC + it, :], y_t)

        # software pipeline: attnV(ic-1) sits between the two score halves
        # of chunk ic so PE has ready work while W8(ic) is produced
        half1 = list(range(NJJ // 2))
        half2 = list(range(NJJ // 2, NJJ))
        W8_cur = w8p.tile([P, NT, SC], fp8, tag="w8")
        pl1 = emit_scores(0, half1)
        emit_elementwise(0, pl1, W8_cur, half1)
        pl2 = emit_scores(0, half2)
        emit_elementwise(0, pl2, W8_cur, half2)
        for ic in range(NC_):
            if ic + 1 < NC_:
                W8_nxt = w8p.tile([P, NT, SC], fp8, tag="w8")
                pl1 = emit_scores(ic + 1, half1)
                emit_elementwise(ic + 1, pl1, W8_nxt, half1)
                emit_attnv(ic, W8_cur)
                pl2 = emit_scores(ic + 1, half2)
                emit_elementwise(ic + 1, pl2, W8_nxt, half2)
                W8_cur = W8_nxt
            else:
                emit_attnv(ic, W8_cur)
        if dbg is not None:
            nc.sync.dma_start(dbg["e"], Es.rearrange("p a b c -> p (a b c)"))
            nc.sync.dma_start(dbg["qh"], qhT.rearrange("p a b -> p (a b)"))
            nc.sync.dma_start(dbg["ql"], qlT.rearrange("p a b -> p (a b)"))
            nc.sync.dma_start(dbg["k"], kT.rearrange("p a b -> p (a b)"))
            nc.sync.dma_start(dbg["vh"], v8h.rearrange("p a b -> p (a b)"))
            nc.sync.dma_start(dbg["vl"], v8l.rearrange("p a b -> p (a b)"))
            nc.sync.dma_start(dbg["cs"], colsum_row.bitcast(f32))
            nc.sync.dma_start(dbg["w8"], W8_cur.rearrange("p a b -> p (a b)"))
        main_ps.__exit__(None, None, None)
        main_pl.__exit__(None, None, None)
        main_po.__exit__(None, None, None)


def build_bass(n: int = 2048, debug: bool = False) -> bass.Bass:
    nc = bacc.Bacc(None, target_bir_lowering=False)
    x = nc.dram_tensor("x", [n, H], f32, kind="ExternalInput")[:, :]
    g = nc.dram_tensor("g", [n, 3], f32, kind="ExternalInput")[:, :]
    wqkv = nc.dram_tensor("w_qkv", [H, 3 * H], f32, kind="ExternalInput")[:, :]
    bqkv = nc.dram_tensor("b_qkv", [3 * H], f32, kind="ExternalInput")[:]
    wout = nc.dram_tensor("w_out", [H, H], f32, kind="ExternalInput")[:, :]
    bout = nc.dram_tensor("b_out", [H], f32, kind="ExternalInput")[:]
    y = nc.dram_tensor("y", [n, H], f32, kind="ExternalOutput")[:, :]
    dbg = None
    if debug:
        NT = n // P
        mk = lambda nm, shape, dt: nc.dram_tensor(
            nm, shape, dt, kind="ExternalOutput")[:, :]
        dbg = {
            "e": mk("dbg_e", [P, (n // SC) * NT * SC], f16),
            "qh": mk("dbg_qh", [P, HT * n], fp8),
            "ql": mk("dbg_ql", [P, HT * n], fp8),
            "k": mk("dbg_k", [P, HT * n], fp8),
            "vh": mk("dbg_vh", [P, NT * H], fp8),
            "vl": mk("dbg_vl", [P, NT * H], fp8),
            "cs": mk("dbg_cs", [1, H], f32),
            "w8": mk("dbg_w8", [P, NT * SC], fp8),
            "ht8": nc.dram_tensor("dbg_ht8", [8, n], f16,
                                  kind="ExternalOutput")[:, :],
            "gt8": nc.dram_tensor("dbg_gt8", [8, n], f16,
                                  kind="ExternalOutput")[:, :],
        }
    with tile.TileContext(nc) as tc:
        _body(tc, n, x, g, wqkv, bqkv, wout, bout, y, dbg=dbg)
    nc.finalize()
    return nc


_CACHED = {}


def _get_nc(n: int = 2048) -> bass.Bass:
    if n not in _CACHED:
        _CACHED[n] = build_bass(n)
    return _CACHED[n]


def kernel(**inputs) -> np.ndarray:
    from concourse.bass_utils import run_bass_kernel_spmd

    x = np.ascontiguousarray(inputs["x"], dtype=np.float32)
    g = np.ascontiguousarray(inputs["geometric_features"], dtype=np.float32)
    wqkv = np.ascontiguousarray(inputs["W_qkv"], dtype=np.float32)
    bqkv = np.ascontiguousarray(inputs["b_qkv"], dtype=np.float32)
    wout = np.ascontiguousarray(inputs["W_out"], dtype=np.float32)
    bout = np.ascontiguousarray(inputs["b_out"], dtype=np.float32)

    B, n, _ = x.shape
    nc = _get_nc(n)
    core_ids = list(range(B))
    in_maps = [
        {
            "x": np.ascontiguousarray(x[b]),
            "g": np.ascontiguousarray(g[b]),
            "w_qkv": wqkv,
            "b_qkv": bqkv,
            "w_out": wout,
            "b_out": bout,
        }
        for b in range(B)
    ]
    res = run_bass_kernel_spmd(nc, in_maps, core_ids)
    return np.stack([res.results[b]["y"] for b in range(B)]).astype(np.float32)


# revision 7
# speedup vs baseline: 1.5287x; 1.0269x over previous
"""EquivariantAttention Trainium2 kernel, v2.

B=8 batches data-parallel over 8 NeuronCores; per core (n=2048, H=512):

  qkv = x @ W_qkv + b ; only q,k,v'' kept where v'' = x @ (W_v@W_out) + b''
  (W_out folded into V so the final projection disappears; b'' = b_v@W_out+b_out
   and the +b_out part rides the softmax identity sum_j p_ij = 1).

  E = exp(-sqrt(d2 + eps)) precomputed f16 for ALL i-chunks in the prologue
  (one Sqrt table block + one Exp table block = 2 ACT table loads total);
  d2 computed on PE from augmented geometry, sqrt reads PSUM directly
  (eps bias replaces the DVE clamp).

  scores S^T[j,i] = k8^T q8 via fp8e4 DoubleRow matmuls (K=256/instr,
  0.5 cycles/row); q split hi+lo fp8 for precision, k single fp8.
  U = exp(S^T*E/sqrt(H)) as f16, W = U-1 quantized e4m3 (near-1 precision),
  attnV natural layout: out[i,h] = colsum(v'') + W8 @ (v_hi8 + v_lo8) via
  DoubleRow; rowsums l = 2048 + W8 @ ones via ap=1 DR matmuls; y = out/l.

Engine split: PE matmuls; ACT sqrt/exp/ynorm; DVE psum copybacks + s*E;
Pool (SBUF-only operands for HW safety) dtype converts + W=U-1 subtract.
"""

import numpy as np

import concourse.bass as bass
from concourse import bacc
import concourse.mybir as mybir
import concourse.tile as tile
from concourse.masks import make_identity
from concourse.tile import add_dep_helper

P = 128
H = 512
SC = 512
HT = H // P  # 4

f32 = mybir.dt.float32
f32r = mybir.dt.float32r
bf16 = mybir.dt.bfloat16
f16 = mybir.dt.float16
fp8 = mybir.dt.float8e4
AF = mybir.ActivationFunctionType
OP = mybir.AluOpType
DR = mybir.MatmulPerfMode.DoubleRow
RSQRT_H = 1.0 / float(np.sqrt(H))
D2_EPS = 1e-4
EPS_OVERRIDE = [None]


def _body(tc, n, x, g, wqkv, bqkv, wout, bout, y, dbg=None):
    nc = tc.nc
    NT = n // P           # j-tiles (16)
    NC_ = n // SC         # i-chunks (4)
    ITC = SC // P         # i-tiles per chunk (4)
    NH = 2                # x-DMA chunk, in units of nt
    NJJ = NT // 2         # j pair-tiles (8)

    state = {"prev": None}

    def chain(a):
        # keep ACT in emission order so table-set switches stay batched
        if state["prev"] is not None:
            add_dep_helper(a.ins, state["prev"].ins, sync=False,
                           reason="ACT table-set batching")
        state["prev"] = a
        return a

    with (
        nc.allow_low_precision(
            reason="fp8/bf16 attention; fp32r transposes and d2"
        ),
        tc.tile_pool(name="const", bufs=1) as const,
        tc.tile_pool(name="attn", bufs=1) as attn,
        tc.tile_pool(name="etp", bufs=2) as etp,
        tc.tile_pool(name="u16p", bufs=2) as u16p,
        tc.tile_pool(name="w8p", bufs=2) as w8p,
        tc.tile_pool(name="yp", bufs=2) as yp,
        tc.tile_pool(name="lp", bufs=2) as lp,
    ):
        # ---------------- constants ----------------
        ident = const.tile([P, P], f32)
        make_identity(nc, ident)
        eps_ap = const.tile([P, 1], f32)
        nc.vector.memset(eps_ap, EPS_OVERRIDE[0] or D2_EPS)
        neg1_ap = const.tile([P, 1], f32)
        nc.vector.memset(neg1_ap, -1.0)
        bqk_sb = const.tile([P, 8], f32)  # cols 0-3 b_q m-tiles, 4-7 b_k
        nc.sync.dma_start(bqk_sb, bqkv[0 : 2 * H].rearrange("(mt p) -> p mt", p=P))
        ones8 = const.tile([P, 2, 1], fp8)
        nc.vector.memset(ones8, 1.0)
        ones_row = const.tile([1, P], f32r)
        nc.vector.memset(ones_row.bitcast(f32), 1.0)
        cN = const.tile([1, ITC], f32r)
        nc.vector.memset(cN.bitcast(f32), float(n))
        colsum_row = const.tile([1, H], f32r)  # written in prologue

        # ---------------- persistent attention tiles ----------------
        qhT = attn.tile([P, HT, n], fp8)   # q hi, [h, i] natural scale
        qlT = attn.tile([P, HT, n], fp8)   # q lo residual
        kT = attn.tile([P, HT, n], fp8)    # k, [h, j]
        Es = attn.tile([P, NC_, NT, SC], f16)  # exp(-dist), [j, i]
        v8h = attn.tile([P, NT, H], fp8)   # v'' hi, [j, h]
        v8l = attn.tile([P, NT, H], fp8)   # v'' lo

        # ================= prologue =================
        with (
            tc.tile_pool(name="geo", bufs=1) as geo,
            tc.tile_pool(name="wsb", bufs=1) as wsb,
            tc.tile_pool(name="xsb", bufs=3) as xsb,
            tc.tile_pool(name="xtc", bufs=2) as xtc,
            tc.tile_pool(name="vt", bufs=1) as vt,
            tc.tile_pool(name="ps_q", bufs=3, space="PSUM") as ps_q,  # 3x1
            tc.tile_pool(name="ps_d", bufs=2, space="PSUM") as ps_d,  # 2x2
            tc.tile_pool(name="ps_one", bufs=1, space="PSUM") as ps_one,  # 1
        ):
            # -- DMA priority order (single DMA engine serializes): g, x0,
            # Wv+Wout pieces (W' needed early), then x chunks alternating
            # with remaining W_qkv pieces; weights stage f32->bf16 on Pool.
            WPC = 128
            g_sb = wsb.tile([P, NT, 3], f32)
            nc.sync.dma_start(g_sb, g.rearrange("(nt p) c -> p nt c", p=P))
            x_r = x.rearrange("(nt p) h -> p nt h", p=P)
            wq_r = wqkv.rearrange("(kt p) m -> p kt m", p=P)
            wo_r = wout.rearrange("(kt p) m -> p kt m", p=P)
            wqkv_bf = wsb.tile([P, HT, 3 * H], bf16)
            wout_bf = wsb.tile([P, HT, H], bf16)

            def stage_w(dst_bf, src_ap):
                # DMA issue on the (otherwise idle) SP queue so the Pool
                # converts don't serialize the staging DMAs behind them
                ws = xsb.tile([P, HT, WPC], f32, tag="ws")
                nc.sync.dma_start(ws, src_ap)
                nc.gpsimd.tensor_copy(dst_bf, ws)

            x_stage = []

            def dma_x(i):
                xs = xsb.tile([P, NH, H], f32, tag="xs")
                nc.sync.dma_start(xs, x_r[:, i * NH : (i + 1) * NH, :])
                x_stage.append(xs)

            for i in range(min(4, NT // NH)):
                dma_x(i)
            for pc in range(8, 12):  # Wv
                stage_w(wqkv_bf[:, :, pc * WPC : (pc + 1) * WPC],
                        wq_r[:, :, pc * WPC : (pc + 1) * WPC])
            for pc in range(4):  # Wout
                stage_w(wout_bf[:, :, pc * WPC : (pc + 1) * WPC],
                        wo_r[:, :, pc * WPC : (pc + 1) * WPC])
            nxt = [0]

            def stage_wq4():
                for _ in range(4):
                    pc = nxt[0]
                    if pc < 8:
                        stage_w(wqkv_bf[:, :, pc * WPC : (pc + 1) * WPC],
                                wq_r[:, :, pc * WPC : (pc + 1) * WPC])
                        nxt[0] = pc + 1

            for i in range(4, NT // NH):
                if i in (4, 5):
                    stage_wq4()
                dma_x(i)
            while nxt[0] < 8:
                stage_wq4()
            bv_sb = wsb.tile([P, HT], f32)
            nc.gpsimd.dma_start(
                bv_sb, bqkv[2 * H : 3 * H].rearrange("(kt p) -> p kt", p=P)
            )
            bpp_row = wsb.tile([1, H], f32r)
            nc.gpsimd.dma_start(
                bpp_row,
                bout.rearrange("(one h) -> one h", one=1).bitcast(f32r))
            bv_bf = wsb.tile([P, HT], bf16)
            nc.gpsimd.tensor_copy(bv_bf, bv_sb)

            # -- augmented geometry in EXACT f16: d2 = |g16_i - g16_j|^2
            # via one K=8 f16 matmul; f16xf16 products are exact in the f32
            # accumulator so d2 >= ~-1e-5 and sqrt(d2+eps) never NaNs on HW
            # (f32r-rounded operands gave d2 errors of +-6e-3 -> NaNs).
            # lhsT rows: [-2g(3), sq_hi, sq_lo, 1, 1, 0]
            # rhs  rows: [ g(3),  1,    1,     sq_hi, sq_lo, 0]
            ident16 = wsb.tile([P, P], f16)
            nc.vector.tensor_copy(ident16, ident)
            ident_bf = wsb.tile([P, P], bf16)
            nc.vector.tensor_copy(ident_bf, ident)
            hT8 = geo.tile([8, n], f16)
            gT8 = geo.tile([8, n], f16)
            g16 = wsb.tile([P, NT, 3], f16)
            nc.vector.tensor_copy(g16, g_sb)
            g2 = wsb.tile([P, NT, 3], f32)
            nc.vector.tensor_mul(g2, g16, g16)
            sq = wsb.tile([P, NT, 1], f32)
            nc.vector.reduce_sum(sq, g2, axis=mybir.AxisListType.X)
            sqh = wsb.tile([P, NT, 1], f16)
            nc.vector.tensor_copy(sqh, sq)
            sql = wsb.tile([P, NT, 1], f16)
            nc.vector.tensor_sub(sql, sq, sqh)
            Ag = wsb.tile([P, NT, 8], f16)
            Ah = wsb.tile([P, NT, 8], f16)
            nc.vector.memset(Ag, 0.0)
            nc.vector.memset(Ah, 0.0)
            nc.vector.tensor_copy(Ag[:, :, 0:3], g16)
            nc.vector.memset(Ag[:, :, 3:5], 1.0)
            nc.vector.tensor_copy(Ag[:, :, 5:6], sqh)
            nc.vector.tensor_copy(Ag[:, :, 6:7], sql)
            nc.vector.tensor_scalar_mul(Ah[:, :, 0:3], g16, -2.0)
            nc.vector.tensor_copy(Ah[:, :, 3:4], sqh)
            nc.vector.tensor_copy(Ah[:, :, 4:5], sql)
            nc.vector.memset(Ah[:, :, 5:7], 1.0)
            for q4 in range(NT // 4):
                for srcA, dstT in ((Ah, hT8), (Ag, gT8)):
                    pq32 = ps_q.tile([P, 4, P], f32, tag="psq")
                    pq = pq32.bitcast(f16)  # [P, 4, 2P]
                    for j in range(4):
                        nt = q4 * 4 + j
                        nc.tensor.transpose(
                            pq[:8, j, 0:P], srcA[:, nt, :], ident16
                        )
                    nc.vector.tensor_copy(
                        dstT[:, q4 * 4 * P : (q4 + 1) * 4 * P],
                        pq[:8, :, 0:P],
                    )

            def emit_d2_chunk(ic):
                # d2 for i-chunk ic: pair-tiles -> psd -> ACT sqrt -> Es f16
                isl = slice(ic * SC, (ic + 1) * SC)
                for jj in range(NJJ):
                    psd = ps_d.tile([P, 2, SC], f32, tag="psd")
                    for s2 in range(2):
                        jt = jj * 2 + s2
                        nc.tensor.matmul(
                            psd[:, s2, :],
                            lhsT=hT8[:, jt * P : (jt + 1) * P],
                            rhs=gT8[:, isl],
                            start=True,
                            stop=True,
                        )
                    chain(
                        nc.scalar.activation(
                            Es[:, ic, jj * 2 : jj * 2 + 2, :],
                            psd,
                            AF.Sqrt,
                            bias=eps_ap,
                        )
                    )

            # -- per n-chunk: x transposes -> xTc (bf16), d2, v'', q/k proj
            # (proj delayed one chunk so its W_qkv pieces have arrived)
            Wp = wsb.tile([P, HT, H], bf16)  # W' = Wv @ Wout, k on partitions
            WvT = wsb.tile([P, HT, H], bf16)  # Wv^T: [h', k]
            pcs = ps_one.tile([1, H], f32, tag="pcs")
            xtc_tiles = {}

            def emit_wprep():
                for kt in range(HT):
                    pq32 = ps_q.tile([P, HT, P], f32, tag="psq")
                    pq = pq32.bitcast(bf16)  # [P, HT, 2*P]
                    for ht in range(HT):
                        nc.tensor.transpose(
                            pq[:, ht, 0:P],
                            wqkv_bf[:, kt,
                                    2 * H + ht * P : 2 * H + (ht + 1) * P],
                            ident_bf,
                        )
                    nc.vector.tensor_copy(
                        WvT[:, :, kt * P : (kt + 1) * P], pq[:, :, 0:P]
                    )
                for kt in range(HT):
                    psp = ps_q.tile([P, 4, P], f32, tag="psq")
                    for hp in range(HT):
                        nc.tensor.matmul(
                            psp,
                            lhsT=WvT[:, hp, kt * P : (kt + 1) * P],
                            rhs=wout_bf[:, hp, :],
                            start=(hp == 0),
                            stop=(hp == HT - 1),
                        )
                    nc.vector.tensor_copy(Wp[:, kt, :], psp)
                # b'' = b_v @ W_out + b_out, broadcast to [P, H]
                psb4 = ps_q.tile([P, 4, P], f32, tag="psq")
                psb = psb4[0:1, :, :]
                for kt in range(HT):
                    nc.tensor.matmul(
                        psb,
                        lhsT=bv_bf[:, kt : kt + 1],
                        rhs=wout_bf[:, kt, :],
                        start=(kt == 0),
                        stop=(kt == HT - 1),
                    )
                nc.vector.tensor_add(bpp_row, psb, bpp_row)

            def emit_vchunk(cv):
                xTc = xtc_tiles[cv]
                for ntl in range(ITC):
                    nt = cv * ITC + ntl
                    psp = ps_q.tile([P, 4, P], f32, tag="psq")
                    nc.tensor.matmul(
                        psp,
                        lhsT=ones_row,
                        rhs=bpp_row,
                        start=True,
                        stop=False,
                        skip_group_check=True,
                    )
                    for kc in range(HT):
                        nc.tensor.matmul(
                            psp,
                            lhsT=xTc[:, kc, ntl * P : (ntl + 1) * P],
                            rhs=Wp[:, kc, :],
                            start=False,
                            stop=(kc == HT - 1),
                            skip_group_check=True,
                        )
                    vtmp = vt.tile([P, H], f32, tag="vt")
                    nc.vector.tensor_copy(vtmp, psp)
                    nc.gpsimd.tensor_copy(v8h[:, nt, :], vtmp)
                    nc.vector.tensor_sub(v8l[:, nt, :], vtmp, v8h[:, nt, :])

            def emit_qkproj(c):
                csl = slice(c * SC, (c + 1) * SC)
                xTc = xtc_tiles.pop(c)
                for mt in [0, 1, 2, 3, 4, 5, 6, 7]:
                    mi = mt % 4
                    psp = ps_q.tile([P, 4, P], f32, tag="psq")
                    for kc in range(HT):
                        nc.tensor.matmul(
                            psp,
                            lhsT=wqkv_bf[:, kc, mt * P : (mt + 1) * P],
                            rhs=xTc[:, kc, :],
                            start=(kc == 0),
                            stop=(kc == HT - 1),
                        )
                    if mt >= 4:
                        nc.vector.tensor_scalar_add(
                            kT[:, mi, csl], psp, bqk_sb[:, 4 + mi : 5 + mi]
                        )
                    else:
                        nc.vector.tensor_scalar_add(
                            qhT[:, mi, csl], psp, bqk_sb[:, mi : mi + 1]
                        )
                        nc.vector.scalar_tensor_tensor(
                            qlT[:, mi, csl],
                            psp,
                            bqk_sb[:, mi : mi + 1],
                            qhT[:, mi, csl],
                            OP.add,
                            OP.subtract,
                        )

            for c in range(NC_):
                xTc = xtc.tile([P, HT, SC], bf16, tag="xtc")
                xtc_tiles[c] = xTc
                for ht in range(HT):
                    pq = ps_q.tile([P, 4, P], f32, tag="psq")
                    for j in range(4):
                        xs = x_stage[2 * c + j // NH]
                        nc.tensor.transpose(
                            pq[:, j, :],
                            xs[:, j % NH, ht * P : (ht + 1) * P],
                            ident,
                        )
                    nc.vector.tensor_copy(xTc[:, ht, :], pq)
                emit_d2_chunk(c)
                if c == min(1, NC_ - 1):
                    emit_wprep()
                for cv in ([c - 1] if c >= 1 else ([] if NC_ > 1 else [0])):
                    emit_vchunk(cv)
                if c >= 1:
                    emit_qkproj(c - 1)
            if NC_ > 1:
                emit_vchunk(NC_ - 1)
            emit_qkproj(NC_ - 1)
            if dbg is not None:
                nc.sync.dma_start(dbg["ht8"], hT8)
                nc.sync.dma_start(dbg["gt8"], gT8)
            # column sums of v'' from the fp8 hi/lo pair (plain fp8 matmuls;
            # DoubleRow with a 1-partition output breaks walrus codegen)
            ones81 = const.tile([P, 1], fp8)
            nc.vector.memset(ones81, 1.0)
            for v8 in (v8h, v8l):
                for jt in range(NT):
                    nc.tensor.matmul(
                        pcs,
                        lhsT=ones81,
                        rhs=v8[:, jt, :],
                        start=(v8 is v8h and jt == 0),
                        stop=(v8 is v8l and jt == NT - 1),
                    )
            nc.vector.tensor_copy(colsum_row, pcs)

            # E = exp(-dist), in place on Es (f16), 4 j-tiles per op;
            # then pin the diagonal back to exactly 1 (E<=1 everywhere, and
            # sqrt(d2+eps) biased the self-distance) via max with identity
            for ic in range(NC_):
                for jq in range(NT // 4):
                    chain(
                        nc.scalar.activation(
                            Es[:, ic, jq * 4 : (jq + 1) * 4, :],
                            Es[:, ic, jq * 4 : (jq + 1) * 4, :],
                            AF.Exp,
                            scale=-1.0,
                        )
                    )
                for it in range(ITC):
                    jd = ic * ITC + it
                    dsl = Es[:, ic, jd, it * P : (it + 1) * P]
                    nc.vector.tensor_max(dsl, dsl, ident16)

        # ================= main attention loop =================
        # open order matters: ps_o/ps_l land on the prologue's psq banks
        # (drained late by the last copybacks), ps_s lands on the d2 banks
        # (drained early) so scores c0 isn't gated on the prologue tail
        main_po = tc.tile_pool(name="ps_o", bufs=2, space="PSUM")
        ps_o = main_po.__enter__()
        main_pl = tc.tile_pool(name="ps_l", bufs=1, space="PSUM")
        ps_l = main_pl.__enter__()
        main_ps = tc.tile_pool(name="ps_s", bufs=2, space="PSUM")
        ps_s = main_ps.__enter__()
        y_r = y.rearrange("(nt p) h -> p nt h", p=P)

        def emit_scores(ic, jjs=None):
            isl = slice(ic * SC, (ic + 1) * SC)
            pss_list = []
            for jj in (jjs if jjs is not None else range(NJJ)):
                pss = ps_s.tile([P, 2, SC], f32, tag="pss")
                for s2 in range(2):
                    jt = jj * 2 + s2
                    jsl = slice(jt * P, (jt + 1) * P)
                    first = True
                    for qT in (qhT, qlT):
                        for kcp in range(HT // 2):
                            nc.tensor.matmul(
                                pss[:, s2, :],
                                lhsT=kT[:, 2 * kcp : 2 * kcp + 2, jsl],
                                rhs=qT[:, 2 * kcp : 2 * kcp + 2, isl],
                                start=first,
                                stop=(qT is qlT) and kcp == HT // 2 - 1,
                                perf_mode=DR,
                            )
                            first = False
                pss_list.append(pss)
            return pss_list

        def emit_elementwise(ic, pss_list, W8, jjs):
            # s*E on DVE, U=exp on ACT, W8=U-1 on Pool
            for i, jj in enumerate(jjs):
                et = etp.tile([P, 2, SC], f16, tag="et")
                nc.vector.tensor_mul(
                    et, pss_list[i], Es[:, ic, jj * 2 : jj * 2 + 2, :]
                )
                u16 = u16p.tile([P, 2, SC], f16, tag="u16")
                chain(nc.scalar.activation(u16, et, AF.Exp, scale=RSQRT_H))
                eng = nc.vector if jj % 4 == 3 else nc.gpsimd
                eng.tensor_scalar(
                    W8[:, jj * 2 : jj * 2 + 2, :], u16, 1.0, 400.0,
                    OP.subtract, OP.min,
                )

        def emit_attnv(ic, W8):
            # rowsums l = n + sum_j W8 (ap=1 DR matmuls into psl columns)
            psl = ps_l.tile([P, ITC], f32, tag="psl")
            for jj in range(NJJ):
                for it in range(ITC):
                    # start only on the very first matmul: the PSUM zero
                    # region is the whole 2KB bank, so a second start would
                    # wipe the other columns' partial sums
                    nc.tensor.matmul(
                        psl[:, it : it + 1],
                        lhsT=W8[:, jj * 2 : jj * 2 + 2, it * P : (it + 1) * P],
                        rhs=ones8,
                        start=(jj == 0 and it == 0),
                        stop=False,
                        perf_mode=DR,
                        skip_group_check=True,
                    )
            nc.tensor.matmul(
                psl,
                lhsT=ones_row,
                rhs=cN,
                start=False,
                stop=True,
                skip_group_check=True,
            )
            linv = lp.tile([P, ITC], f32, tag="linv")
            nc.vector.reciprocal(linv, psl)
            # out[i, h] = colsum + W8 @ (v_hi + v_lo), then y = out * (1/l)
            for it in range(ITC):
                pso = ps_o.tile([P, SC], f32, tag="pso")
                nc.tensor.matmul(
                    pso,
                    lhsT=ones_row,
                    rhs=colsum_row,
                    start=True,
                    stop=False,
                    skip_group_check=True,
                )
                itsl = slice(it * P, (it + 1) * P)
                for v8 in (v8h, v8l):
                    for jj in range(NJJ):
                        nc.tensor.matmul(
                            pso,
                            lhsT=W8[:, jj * 2 : jj * 2 + 2, itsl],
                            rhs=v8[:, jj * 2 : jj * 2 + 2, :],
                            start=False,
                            stop=(v8 is v8l) and jj == NJJ - 1,
                            perf_mode=DR,
                            skip_group_check=True,
                        )
                y_t = yp.tile([P, H], f32, tag="y")
                nc.vector.tensor_scalar_mul(y_t, pso, linv[:, it : it + 1])
                nc.sync.dma_start(y_r[:, ic * ITC + it, :], y_t)

        # software pipeline: attnV(ic-1) sits between the two score halves
        # of chunk ic so PE has ready work while W8(ic) is produced
        half1 = list(range(NJJ // 2))
        half2 = list(range(NJJ // 2, NJJ))
        W8_cur = w8p.tile([P, NT, SC], fp8, tag="w8")
        pl1 = emit_scores(0, half1)
        emit_elementwise(0, pl1, W8_cur, half1)
        pl2 = emit_scores(0, half2)
        emit_elementwise(0, pl2, W8_cur, half2)
        for ic in range(NC_):
            if ic + 1 < NC_:
                W8_nxt = w8p.tile([P, NT, SC], fp8, tag="w8")
                pl1 = emit_scores(ic + 1, half1)
                emit_elementwise(ic + 1, pl1, W8_nxt, half1)
                emit_attnv(ic, W8_cur)
                pl2 = emit_scores(ic + 1, half2)
                emit_elementwise(ic + 1, pl2, W8_nxt, half2)
                W8_cur = W8_nxt
            else:
                emit_attnv(ic, W8_cur)
        if dbg is not None:
            nc.sync.dma_start(dbg["e"], Es.rearrange("p a b c -> p (a b c)"))
            nc.sync.dma_start(dbg["qh"], qhT.rearrange("p a b -> p (a b)"))
            nc.sync.dma_start(dbg["ql"], qlT.rearrange("p a b -> p (a b)"))
            nc.sync.dma_start(dbg["k"], kT.rearrange("p a b -> p (a b)"))
            nc.sync.dma_start(dbg["vh"], v8h.rearrange("p a b -> p (a b)"))
            nc.sync.dma_start(dbg["vl"], v8l.rearrange("p a b -> p (a b)"))
            nc.sync.dma_start(dbg["cs"], colsum_row.bitcast(f32))
            nc.sync.dma_start(dbg["w8"], W8_cur.rearrange("p a b -> p (a b)"))
        main_ps.__exit__(None, None, None)
        main_pl.__exit__(None, None, None)
        main_po.__exit__(None, None, None)


def build_bass(n: int = 2048, debug: bool = False) -> bass.Bass:
    nc = bacc.Bacc(None, target_bir_lowering=False)
    x = nc.dram_tensor("x", [n, H], f32, kind="ExternalInput")[:, :]
    g = nc.dram_tensor("g", [n, 3], f32, kind="ExternalInput")[:, :]
    wqkv = nc.dram_tensor("w_qkv", [H, 3 * H], f32, kind="ExternalInput")[:, :]
    bqkv = nc.dram_tensor("b_qkv", [3 * H], f32, kind="ExternalInput")[:]
    wout = nc.dram_tensor("w_out", [H, H], f32, kind="ExternalInput")[:, :]
    bout = nc.dram_tensor("b_out", [H], f32, kind="ExternalInput")[:]
    y = nc.dram_tensor("y", [n, H], f32, kind="ExternalOutput")[:, :]
    dbg = None
    if debug:
        NT = n // P
        mk = lambda nm, shape, dt: nc.dram_tensor(
            nm, shape, dt, kind="ExternalOutput")[:, :]
        dbg = {
            "e": mk("dbg_e", [P, (n // SC) * NT * SC], f16),
            "qh": mk("dbg_qh", [P, HT * n], fp8),
            "ql": mk("dbg_ql", [P, HT * n], fp8),
            "k": mk("dbg_k", [P, HT * n], fp8),
            "vh": mk("dbg_vh", [P, NT * H], fp8),
            "vl": mk("dbg_vl", [P, NT * H], fp8),
            "cs": mk("dbg_cs", [1, H], f32),
            "w8": mk("dbg_w8", [P, NT * SC], fp8),
            "ht8": nc.dram_tensor("dbg_ht8", [8, n], f16,
                                  kind="ExternalOutput")[:, :],
            "gt8": nc.dram_tensor("dbg_gt8", [8, n], f16,
                                  kind="ExternalOutput")[:, :],
        }
    with tile.TileContext(nc) as tc:
        _body(tc, n, x, g, wqkv, bqkv, wout, bout, y, dbg=dbg)
    nc.finalize()
    return nc


_CACHED = {}


def _get_nc(n: int = 2048) -> bass.Bass:
    if n not in _CACHED:
        _CACHED[n] = build_bass(n)
    return _CACHED[n]


def kernel(**inputs) -> np.ndarray:
    from concourse.bass_utils import run_bass_kernel_spmd

    x = np.ascontiguousarray(inputs["x"], dtype=np.float32)
    g = np.ascontiguousarray(inputs["geometric_features"], dtype=np.float32)
    wqkv = np.ascontiguousarray(inputs["W_qkv"], dtype=np.float32)
    bqkv = np.ascontiguousarray(inputs["b_qkv"], dtype=np.float32)
    wout = np.ascontiguousarray(inputs["W_out"], dtype=np.float32)
    bout = np.ascontiguousarray(inputs["b_out"], dtype=np.float32)

    B, n, _ = x.shape
    nc = _get_nc(n)
    core_ids = list(range(B))
    in_maps = [
        {
            "x": np.ascontiguousarray(x[b]),
            "g": np.ascontiguousarray(g[b]),
            "w_qkv": wqkv,
            "b_qkv": bqkv,
            "w_out": wout,
            "b_out": bout,
        }
        for b in range(B)
    ]
    res = run_bass_kernel_spmd(nc, in_maps, core_ids)
    return np.stack([res.results[b]["y"] for b in range(B)]).astype(np.float32)


# revision 8
# speedup vs baseline: 1.5291x; 1.0003x over previous
"""EquivariantAttention Trainium2 kernel, v2.

B=8 batches data-parallel over 8 NeuronCores; per core (n=2048, H=512):

  qkv = x @ W_qkv + b ; only q,k,v'' kept where v'' = x @ (W_v@W_out) + b''
  (W_out folded into V so the final projection disappears; b'' = b_v@W_out+b_out
   and the +b_out part rides the softmax identity sum_j p_ij = 1).

  E = exp(-sqrt(d2 + eps)) precomputed f16 for ALL i-chunks in the prologue
  (one Sqrt table block + one Exp table block = 2 ACT table loads total);
  d2 computed on PE from augmented geometry, sqrt reads PSUM directly
  (eps bias replaces the DVE clamp).

  scores S^T[j,i] = k8^T q8 via fp8e4 DoubleRow matmuls (K=256/instr,
  0.5 cycles/row); q split hi+lo fp8 for precision, k single fp8.
  U = exp(S^T*E/sqrt(H)) as f16, W = U-1 quantized e4m3 (near-1 precision),
  attnV natural layout: out[i,h] = colsum(v'') + W8 @ (v_hi8 + v_lo8) via
  DoubleRow; rowsums l = 2048 + W8 @ ones via ap=1 DR matmuls; y = out/l.

Engine split: PE matmuls; ACT sqrt/exp/ynorm; DVE psum copybacks + s*E;
Pool (SBUF-only operands for HW safety) dtype converts + W=U-1 subtract.
"""

import numpy as np

import concourse.bass as bass
from concourse import bacc
import concourse.mybir as mybir
import concourse.tile as tile
from concourse.masks import make_identity
from concourse.tile import add_dep_helper

P = 128
H = 512
SC = 512
HT = H // P  # 4

f32 = mybir.dt.float32
f32r = mybir.dt.float32r
bf16 = mybir.dt.bfloat16
f16 = mybir.dt.float16
fp8 = mybir.dt.float8e4
AF = mybir.ActivationFunctionType
OP = mybir.AluOpType
DR = mybir.MatmulPerfMode.DoubleRow
RSQRT_H = 1.0 / float(np.sqrt(H))
D2_EPS = 1e-4
EPS_OVERRIDE = [None]


def _body(tc, n, x, g, wqkv, bqkv, wout, bout, y, dbg=None):
    nc = tc.nc
    NT = n // P           # j-tiles (16)
    NC_ = n // SC         # i-chunks (4)
    ITC = SC // P         # i-tiles per chunk (4)
    NH = 2                # x-DMA chunk, in units of nt
    NJJ = NT // 2         # j pair-tiles (8)

    state = {"prev": None}

    def chain(a):
        # keep ACT in emission order so table-set switches stay batched
        if state["prev"] is not None:
            add_dep_helper(a.ins, state["prev"].ins, sync=False,
                           reason="ACT table-set batching")
        state["prev"] = a
        return a

    with (
        nc.allow_low_precision(
            reason="fp8/bf16 attention; fp32r transposes and d2"
        ),
        tc.tile_pool(name="const", bufs=1) as const,
        tc.tile_pool(name="attn", bufs=1) as attn,
        tc.tile_pool(name="etp", bufs=2) as etp,
        tc.tile_pool(name="u16p", bufs=2) as u16p,
        tc.tile_pool(name="w8p", bufs=2) as w8p,
        tc.tile_pool(name="yp", bufs=2) as yp,
        tc.tile_pool(name="lp", bufs=2) as lp,
    ):
        # ---------------- constants ----------------
        ident = const.tile([P, P], f32)
        make_identity(nc, ident)
        eps_ap = const.tile([P, 1], f32)
        nc.vector.memset(eps_ap, EPS_OVERRIDE[0] or D2_EPS)
        neg1_ap = const.tile([P, 1], f32)
        nc.vector.memset(neg1_ap, -1.0)
        bqk_sb = const.tile([P, 8], f32)  # cols 0-3 b_q m-tiles, 4-7 b_k
        nc.sync.dma_start(bqk_sb, bqkv[0 : 2 * H].rearrange("(mt p) -> p mt", p=P))
        ones8 = const.tile([P, 2, 1], fp8)
        nc.vector.memset(ones8, 1.0)
        ones_row = const.tile([1, P], f32r)
        nc.vector.memset(ones_row.bitcast(f32), 1.0)
        cN = const.tile([1, ITC], f32r)
        nc.vector.memset(cN.bitcast(f32), float(n))
        colsum_row = const.tile([1, H], f32r)  # written in prologue

        # ---------------- persistent attention tiles ----------------
        qhT = attn.tile([P, HT, n], fp8)   # q hi, [h, i] natural scale
        qlT = attn.tile([P, HT, n], fp8)   # q lo residual
        kT = attn.tile([P, HT, n], fp8)    # k, [h, j]
        Es = attn.tile([P, NC_, NT, SC], f16)  # exp(-dist), [j, i]
        v8h = attn.tile([P, NT, H], fp8)   # v'' hi, [j, h]
        v8l = attn.tile([P, NT, H], fp8)   # v'' lo

        # ================= prologue =================
        with (
            tc.tile_pool(name="geo", bufs=1) as geo,
            tc.tile_pool(name="wsb", bufs=1) as wsb,
            tc.tile_pool(name="xsb", bufs=3) as xsb,
            tc.tile_pool(name="xtc", bufs=2) as xtc,
            tc.tile_pool(name="vt", bufs=1) as vt,
            tc.tile_pool(name="ps_q", bufs=3, space="PSUM") as ps_q,  # 3x1
            tc.tile_pool(name="ps_d", bufs=2, space="PSUM") as ps_d,  # 2x2
            tc.tile_pool(name="ps_one", bufs=1, space="PSUM") as ps_one,  # 1
        ):
            # -- DMA priority order (single DMA engine serializes): g, x0,
            # Wv+Wout pieces (W' needed early), then x chunks alternating
            # with remaining W_qkv pieces; weights stage f32->bf16 on Pool.
            WPC = 128
            g_sb = wsb.tile([P, NT, 3], f32)
            nc.sync.dma_start(g_sb, g.rearrange("(nt p) c -> p nt c", p=P))
            x_r = x.rearrange("(nt p) h -> p nt h", p=P)
            wq_r = wqkv.rearrange("(kt p) m -> p kt m", p=P)
            wo_r = wout.rearrange("(kt p) m -> p kt m", p=P)
            wqkv_bf = wsb.tile([P, HT, 3 * H], bf16)
            wout_bf = wsb.tile([P, HT, H], bf16)

            def stage_w(dst_bf, src_ap):
                # DMA issue on the (otherwise idle) SP queue so the Pool
                # converts don't serialize the staging DMAs behind them
                ws = xsb.tile([P, HT, WPC], f32, tag="ws")
                nc.sync.dma_start(ws, src_ap)
                nc.gpsimd.tensor_copy(dst_bf, ws)

            x_stage = []

            def dma_x(i):
                xs = xsb.tile([P, NH, H], f32, tag="xs")
                nc.sync.dma_start(xs, x_r[:, i * NH : (i + 1) * NH, :])
                x_stage.append(xs)

            for i in range(min(4, NT // NH)):
                dma_x(i)
            for pc in range(8, 12):  # Wv
                stage_w(wqkv_bf[:, :, pc * WPC : (pc + 1) * WPC],
                        wq_r[:, :, pc * WPC : (pc + 1) * WPC])
            for pc in range(4):  # Wout
                stage_w(wout_bf[:, :, pc * WPC : (pc + 1) * WPC],
                        wo_r[:, :, pc * WPC : (pc + 1) * WPC])
            nxt = [0]

            def stage_wq4():
                for _ in range(4):
                    pc = nxt[0]
                    if pc < 8:
                        stage_w(wqkv_bf[:, :, pc * WPC : (pc + 1) * WPC],
                                wq_r[:, :, pc * WPC : (pc + 1) * WPC])
                        nxt[0] = pc + 1

            for i in range(4, NT // NH):
                if i in (4, 5):
                    stage_wq4()
                dma_x(i)
            while nxt[0] < 8:
                stage_wq4()
            bv_sb = wsb.tile([P, HT], f32)
            nc.gpsimd.dma_start(
                bv_sb, bqkv[2 * H : 3 * H].rearrange("(kt p) -> p kt", p=P)
            )
            bpp_row = wsb.tile([1, H], f32r)
            nc.gpsimd.dma_start(
                bpp_row,
                bout.rearrange("(one h) -> one h", one=1).bitcast(f32r))
            bv_bf = wsb.tile([P, HT], bf16)
            nc.gpsimd.tensor_copy(bv_bf, bv_sb)

            # -- augmented geometry in EXACT f16: d2 = |g16_i - g16_j|^2
            # via one K=8 f16 matmul; f16xf16 products are exact in the f32
            # accumulator so d2 >= ~-1e-5 and sqrt(d2+eps) never NaNs on HW
            # (f32r-rounded operands gave d2 errors of +-6e-3 -> NaNs).
            # lhsT rows: [-2g(3), sq_hi, sq_lo, 1, 1, 0]
            # rhs  rows: [ g(3),  1,    1,     sq_hi, sq_lo, 0]
            ident16 = wsb.tile([P, P], f16)
            nc.vector.tensor_copy(ident16, ident)
            ident_bf = wsb.tile([P, P], bf16)
            nc.vector.tensor_copy(ident_bf, ident)
            hT8 = geo.tile([8, n], f16)
            gT8 = geo.tile([8, n], f16)
            g16 = wsb.tile([P, NT, 3], f16)
            nc.vector.tensor_copy(g16, g_sb)
            g2 = wsb.tile([P, NT, 3], f32)
            nc.vector.tensor_mul(g2, g16, g16)
            sq = wsb.tile([P, NT, 1], f32)
            nc.vector.reduce_sum(sq, g2, axis=mybir.AxisListType.X)
            sqh = wsb.tile([P, NT, 1], f16)
            nc.vector.tensor_copy(sqh, sq)
            sql = wsb.tile([P, NT, 1], f16)
            nc.vector.tensor_sub(sql, sq, sqh)
            Ag = wsb.tile([P, NT, 8], f16)
            Ah = wsb.tile([P, NT, 8], f16)
            nc.vector.memset(Ag, 0.0)
            nc.vector.memset(Ah, 0.0)
            nc.vector.tensor_copy(Ag[:, :, 0:3], g16)
            nc.vector.memset(Ag[:, :, 3:5], 1.0)
            nc.vector.tensor_copy(Ag[:, :, 5:6], sqh)
            nc.vector.tensor_copy(Ag[:, :, 6:7], sql)
            nc.vector.tensor_scalar_mul(Ah[:, :, 0:3], g16, -2.0)
            nc.vector.tensor_copy(Ah[:, :, 3:4], sqh)
            nc.vector.tensor_copy(Ah[:, :, 4:5], sql)
            nc.vector.memset(Ah[:, :, 5:7], 1.0)
            for q4 in range(NT // 4):
                for srcA, dstT in ((Ah, hT8), (Ag, gT8)):
                    pq32 = ps_q.tile([P, 4, P], f32, tag="psq")
                    pq = pq32.bitcast(f16)  # [P, 4, 2P]
                    for j in range(4):
                        nt = q4 * 4 + j
                        nc.tensor.transpose(
                            pq[:8, j, 0:P], srcA[:, nt, :], ident16
                        )
                    nc.scalar.copy(
                        dstT[:, q4 * 4 * P : (q4 + 1) * 4 * P],
                        pq[:8, :, 0:P],
                    )

            def emit_d2_chunk(ic):
                # d2 for i-chunk ic: pair-tiles -> psd -> ACT sqrt -> Es f16
                isl = slice(ic * SC, (ic + 1) * SC)
                for jj in range(NJJ):
                    psd = ps_d.tile([P, 2, SC], f32, tag="psd")
                    for s2 in range(2):
                        jt = jj * 2 + s2
                        nc.tensor.matmul(
                            psd[:, s2, :],
                            lhsT=hT8[:, jt * P : (jt + 1) * P],
                            rhs=gT8[:, isl],
                            start=True,
                            stop=True,
                        )
                    chain(
                        nc.scalar.activation(
                            Es[:, ic, jj * 2 : jj * 2 + 2, :],
                            psd,
                            AF.Sqrt,
                            bias=eps_ap,
                        )
                    )

            # -- per n-chunk: x transposes -> xTc (bf16), d2, v'', q/k proj
            # (proj delayed one chunk so its W_qkv pieces have arrived)
            Wp = wsb.tile([P, HT, H], bf16)  # W' = Wv @ Wout, k on partitions
            WvT = wsb.tile([P, HT, H], bf16)  # Wv^T: [h', k]
            pcs = ps_one.tile([1, H], f32, tag="pcs")
            xtc_tiles = {}

            def emit_wprep():
                for kt in range(HT):
                    pq32 = ps_q.tile([P, HT, P], f32, tag="psq")
                    pq = pq32.bitcast(bf16)  # [P, HT, 2*P]
                    for ht in range(HT):
                        nc.tensor.transpose(
                            pq[:, ht, 0:P],
                            wqkv_bf[:, kt,
                                    2 * H + ht * P : 2 * H + (ht + 1) * P],
                            ident_bf,
                        )
                    nc.vector.tensor_copy(
                        WvT[:, :, kt * P : (kt + 1) * P], pq[:, :, 0:P]
                    )
                for kt in range(HT):
                    psp = ps_q.tile([P, 4, P], f32, tag="psq")
                    for hp in range(HT):
                        nc.tensor.matmul(
                            psp,
                            lhsT=WvT[:, hp, kt * P : (kt + 1) * P],
                            rhs=wout_bf[:, hp, :],
                            start=(hp == 0),
                            stop=(hp == HT - 1),
                        )
                    nc.vector.tensor_copy(Wp[:, kt, :], psp)
                # b'' = b_v @ W_out + b_out, broadcast to [P, H]
                psb4 = ps_q.tile([P, 4, P], f32, tag="psq")
                psb = psb4[0:1, :, :]
                for kt in range(HT):
                    nc.tensor.matmul(
                        psb,
                        lhsT=bv_bf[:, kt : kt + 1],
                        rhs=wout_bf[:, kt, :],
                        start=(kt == 0),
                        stop=(kt == HT - 1),
                    )
                nc.vector.tensor_add(bpp_row, psb, bpp_row)

            def emit_vchunk(cv):
                xTc = xtc_tiles[cv]
                for ntl in range(ITC):
                    nt = cv * ITC + ntl
                    psp = ps_q.tile([P, 4, P], f32, tag="psq")
                    nc.tensor.matmul(
                        psp,
                        lhsT=ones_row,
                        rhs=bpp_row,
                        start=True,
                        stop=False,
                        skip_group_check=True,
                    )
                    for kc in range(HT):
                        nc.tensor.matmul(
                            psp,
                            lhsT=xTc[:, kc, ntl * P : (ntl + 1) * P],
                            rhs=Wp[:, kc, :],
                            start=False,
                            stop=(kc == HT - 1),
                            skip_group_check=True,
                        )
                    vtmp = vt.tile([P, H], f32, tag="vt")
                    nc.vector.tensor_copy(vtmp, psp)
                    nc.gpsimd.tensor_copy(v8h[:, nt, :], vtmp)
                    nc.vector.tensor_sub(v8l[:, nt, :], vtmp, v8h[:, nt, :])

            def emit_qkproj(c):
                csl = slice(c * SC, (c + 1) * SC)
                xTc = xtc_tiles.pop(c)
                for mt in [0, 1, 2, 3, 4, 5, 6, 7]:
                    mi = mt % 4
                    psp = ps_q.tile([P, 4, P], f32, tag="psq")
                    for kc in range(HT):
                        nc.tensor.matmul(
                            psp,
                            lhsT=wqkv_bf[:, kc, mt * P : (mt + 1) * P],
                            rhs=xTc[:, kc, :],
                            start=(kc == 0),
                            stop=(kc == HT - 1),
                        )
                    if mt >= 4:
                        nc.vector.tensor_scalar_add(
                            kT[:, mi, csl], psp, bqk_sb[:, 4 + mi : 5 + mi]
                        )
                    else:
                        nc.vector.tensor_scalar_add(
                            qhT[:, mi, csl], psp, bqk_sb[:, mi : mi + 1]
                        )
                        nc.vector.scalar_tensor_tensor(
                            qlT[:, mi, csl],
                            psp,
                            bqk_sb[:, mi : mi + 1],
                            qhT[:, mi, csl],
                            OP.add,
                            OP.subtract,
                        )

            for c in range(NC_):
                xTc = xtc.tile([P, HT, SC], bf16, tag="xtc")
                xtc_tiles[c] = xTc
                for ht in range(HT):
                    pq = ps_q.tile([P, 4, P], f32, tag="psq")
                    for j in range(4):
                        xs = x_stage[2 * c + j // NH]
                        nc.tensor.transpose(
                            pq[:, j, :],
                            xs[:, j % NH, ht * P : (ht + 1) * P],
                            ident,
                        )
                    nc.vector.tensor_copy(xTc[:, ht, :], pq)
                emit_d2_chunk(c)
                if c == min(1, NC_ - 1):
                    emit_wprep()
                for cv in ([c - 1] if c >= 1 else ([] if NC_ > 1 else [0])):
                    emit_vchunk(cv)
                if c >= 1:
                    emit_qkproj(c - 1)
            if NC_ > 1:
                emit_vchunk(NC_ - 1)
            emit_qkproj(NC_ - 1)
            if dbg is not None:
                nc.sync.dma_start(dbg["ht8"], hT8)
                nc.sync.dma_start(dbg["gt8"], gT8)
            # column sums of v'' from the fp8 hi/lo pair (plain fp8 matmuls;
            # DoubleRow with a 1-partition output breaks walrus codegen)
            ones81 = const.tile([P, 1], fp8)
            nc.vector.memset(ones81, 1.0)
            for v8 in (v8h, v8l):
                for jt in range(NT):
                    nc.tensor.matmul(
                        pcs,
                        lhsT=ones81,
                        rhs=v8[:, jt, :],
                        start=(v8 is v8h and jt == 0),
                        stop=(v8 is v8l and jt == NT - 1),
                    )
            nc.vector.tensor_copy(colsum_row, pcs)

            # E = exp(-dist), in place on Es (f16), 4 j-tiles per op;
            # then pin the diagonal back to exactly 1 (E<=1 everywhere, and
            # sqrt(d2+eps) biased the self-distance) via max with identity
            for ic in range(NC_):
                for jq in range(NT // 4):
                    chain(
                        nc.scalar.activation(
                            Es[:, ic, jq * 4 : (jq + 1) * 4, :],
                            Es[:, ic, jq * 4 : (jq + 1) * 4, :],
                            AF.Exp,
                            scale=-1.0,
                        )
                    )
                for it in range(ITC):
                    jd = ic * ITC + it
                    dsl = Es[:, ic, jd, it * P : (it + 1) * P]
                    nc.vector.tensor_max(dsl, dsl, ident16)

        # ================= main attention loop =================
        # open order matters: ps_o/ps_l land on the prologue's psq banks
        # (drained late by the last copybacks), ps_s lands on the d2 banks
        # (drained early) so scores c0 isn't gated on the prologue tail
        main_po = tc.tile_pool(name="ps_o", bufs=2, space="PSUM")
        ps_o = main_po.__enter__()
        main_pl = tc.tile_pool(name="ps_l", bufs=1, space="PSUM")
        ps_l = main_pl.__enter__()
        main_ps = tc.tile_pool(name="ps_s", bufs=2, space="PSUM")
        ps_s = main_ps.__enter__()
        y_r = y.rearrange("(nt p) h -> p nt h", p=P)

        def emit_scores(ic, jjs=None):
            isl = slice(ic * SC, (ic + 1) * SC)
            pss_list = []
            for jj in (jjs if jjs is not None else range(NJJ)):
                pss = ps_s.tile([P, 2, SC], f32, tag="pss")
                for s2 in range(2):
                    jt = jj * 2 + s2
                    jsl = slice(jt * P, (jt + 1) * P)
                    first = True
                    for qT in (qhT, qlT):
                        for kcp in range(HT // 2):
                            nc.tensor.matmul(
                                pss[:, s2, :],
                                lhsT=kT[:, 2 * kcp : 2 * kcp + 2, jsl],
                                rhs=qT[:, 2 * kcp : 2 * kcp + 2, isl],
                                start=first,
                                stop=(qT is qlT) and kcp == HT // 2 - 1,
                                perf_mode=DR,
                            )
                            first = False
                pss_list.append(pss)
            return pss_list

        def emit_elementwise(ic, pss_list, W8, jjs):
            # s*E on DVE, U=exp on ACT, W8=U-1 on Pool
            for i, jj in enumerate(jjs):
                et = etp.tile([P, 2, SC], f16, tag="et")
                nc.vector.tensor_mul(
                    et, pss_list[i], Es[:, ic, jj * 2 : jj * 2 + 2, :]
                )
                u16 = u16p.tile([P, 2, SC], f16, tag="u16")
                chain(nc.scalar.activation(u16, et, AF.Exp, scale=RSQRT_H))
                eng = nc.vector if jj % 4 == 3 else nc.gpsimd
                eng.tensor_scalar(
                    W8[:, jj * 2 : jj * 2 + 2, :], u16, 1.0, 400.0,
                    OP.subtract, OP.min,
                )

        def emit_attnv(ic, W8):
            # rowsums l = n + sum_j W8 (ap=1 DR matmuls into psl columns)
            psl = ps_l.tile([P, ITC], f32, tag="psl")
            for jj in range(NJJ):
                for it in range(ITC):
                    # start only on the very first matmul: the PSUM zero
                    # region is the whole 2KB bank, so a second start would
                    # wipe the other columns' partial sums
                    nc.tensor.matmul(
                        psl[:, it : it + 1],
                        lhsT=W8[:, jj * 2 : jj * 2 + 2, it * P : (it + 1) * P],
                        rhs=ones8,
                        start=(jj == 0 and it == 0),
                        stop=False,
                        perf_mode=DR,
                        skip_group_check=True,
                    )
            nc.tensor.matmul(
                psl,
                lhsT=ones_row,
                rhs=cN,
                start=False,
                stop=True,
                skip_group_check=True,
            )
            linv = lp.tile([P, ITC], f32, tag="linv")
            nc.vector.reciprocal(linv, psl)
            # out[i, h] = colsum + W8 @ (v_hi + v_lo), then y = out * (1/l)
            for it in range(ITC):
                pso = ps_o.tile([P, SC], f32, tag="pso")
                nc.tensor.matmul(
                    pso,
                    lhsT=ones_row,
                    rhs=colsum_row,
                    start=True,
                    stop=False,
                    skip_group_check=True,
                )
                itsl = slice(it * P, (it + 1) * P)
                for v8 in (v8h, v8l):
                    for jj in range(NJJ):
                        nc.tensor.matmul(
                            pso,
                            lhsT=W8[:, jj * 2 : jj * 2 + 2, itsl],
                            rhs=v8[:, jj * 2 : jj * 2 + 2, :],
                            start=False,
                            stop=(v8 is v8l) and jj == NJJ - 1,
                            perf_mode=DR,
                            skip_group_check=True,
                        )
                y_t = yp.tile([P, H], f32, tag="y")
                nc.vector.tensor_scalar_mul(y_t, pso, linv[:, it : it + 1])
                nc.sync.dma_start(y_r[:, ic * ITC + it, :], y_t)

        # software pipeline: attnV(ic-1) sits between the two score halves
        # of chunk ic so PE has ready work while W8(ic) is produced
        half1 = list(range(NJJ // 2))
        half2 = list(range(NJJ // 2, NJJ))
        W8_cur = w8p.tile([P, NT, SC], fp8, tag="w8")
        pl1 = emit_scores(0, half1)
        emit_elementwise(0, pl1, W8_cur, half1)
        pl2 = emit_scores(0, half2)
        emit_elementwise(0, pl2, W8_cur, half2)
        for ic in range(NC_):
            if ic + 1 < NC_:
                W8_nxt = w8p.tile([P, NT, SC], fp8, tag="w8")
                pl1 = emit_scores(ic + 1, half1)
                emit_elementwise(ic + 1, pl1, W8_nxt, half1)
                emit_attnv(ic, W8_cur)
                pl2 = emit_scores(ic + 1, half2)
                emit_elementwise(ic + 1, pl2, W8_nxt, half2)
                W8_cur = W8_nxt
            else:
                emit_attnv(ic, W8_cur)
        if dbg is not None:
            nc.sync.dma_start(dbg["e"], Es.rearrange("p a b c -> p (a b c)"))
            nc.sync.dma_start(dbg["qh"], qhT.rearrange("p a b -> p (a b)"))
            nc.sync.dma_start(dbg["ql"], qlT.rearrange("p a b -> p (a b)"))
            nc.sync.dma_start(dbg["k"], kT.rearrange("p a b -> p (a b)"))
            nc.sync.dma_start(dbg["vh"], v8h.rearrange("p a b -> p (a b)"))
            nc.sync.dma_start(dbg["vl"], v8l.rearrange("p a b -> p (a b)"))
            nc.sync.dma_start(dbg["cs"], colsum_row.bitcast(f32))
            nc.sync.dma_start(dbg["w8"], W8_cur.rearrange("p a b -> p (a b)"))
        main_ps.__exit__(None, None, None)
        main_pl.__exit__(None, None, None)
        main_po.__exit__(None, None, None)


def build_bass(n: int = 2048, debug: bool = False) -> bass.Bass:
    nc = bacc.Bacc(None, target_bir_lowering=False)
    x = nc.dram_tensor("x", [n, H], f32, kind="ExternalInput")[:, :]
    g = nc.dram_tensor("g", [n, 3], f32, kind="ExternalInput")[:, :]
    wqkv = nc.dram_tensor("w_qkv", [H, 3 * H], f32, kind="ExternalInput")[:, :]
    bqkv = nc.dram_tensor("b_qkv", [3 * H], f32, kind="ExternalInput")[:]
    wout = nc.dram_tensor("w_out", [H, H], f32, kind="ExternalInput")[:, :]
    bout = nc.dram_tensor("b_out", [H], f32, kind="ExternalInput")[:]
    y = nc.dram_tensor("y", [n, H], f32, kind="ExternalOutput")[:, :]
    dbg = None
    if debug:
        NT = n // P
        mk = lambda nm, shape, dt: nc.dram_tensor(
            nm, shape, dt, kind="ExternalOutput")[:, :]
        dbg = {
            "e": mk("dbg_e", [P, (n // SC) * NT * SC], f16),
            "qh": mk("dbg_qh", [P, HT * n], fp8),
            "ql": mk("dbg_ql", [P, HT * n], fp8),
            "k": mk("dbg_k", [P, HT * n], fp8),
            "vh": mk("dbg_vh", [P, NT * H], fp8),
            "vl": mk("dbg_vl", [P, NT * H], fp8),
            "cs": mk("dbg_cs", [1, H], f32),
            "w8": mk("dbg_w8", [P, NT * SC], fp8),
            "ht8": nc.dram_tensor("dbg_ht8", [8, n], f16,
                                  kind="ExternalOutput")[:, :],
            "gt8": nc.dram_tensor("dbg_gt8", [8, n], f16,
                                  kind="ExternalOutput")[:, :],
        }
    with tile.TileContext(nc) as tc:
        _body(tc, n, x, g, wqkv, bqkv, wout, bout, y, dbg=dbg)
    nc.finalize()
    return nc


_CACHED = {}


def _get_nc(n: int = 2048) -> bass.Bass:
    if n not in _CACHED:
        _CACHED[n] = build_bass(n)
    return _CACHED[n]


def kernel(**inputs) -> np.ndarray:
    from concourse.bass_utils import run_bass_kernel_spmd

    x = np.ascontiguousarray(inputs["x"], dtype=np.float32)
    g = np.ascontiguousarray(inputs["geometric_features"], dtype=np.float32)
    wqkv = np.ascontiguousarray(inputs["W_qkv"], dtype=np.float32)
    bqkv = np.ascontiguousarray(inputs["b_qkv"], dtype=np.float32)
    wout = np.ascontiguousarray(inputs["W_out"], dtype=np.float32)
    bout = np.ascontiguousarray(inputs["b_out"], dtype=np.float32)

    B, n, _ = x.shape
    nc = _get_nc(n)
    core_ids = list(range(B))
    in_maps = [
        {
            "x": np.ascontiguousarray(x[b]),
            "g": np.ascontiguousarray(g[b]),
            "w_qkv": wqkv,
            "b_qkv": bqkv,
            "w_out": wout,
            "b_out": bout,
        }
        for b in range(B)
    ]
    res = run_bass_kernel_spmd(nc, in_maps, core_ids)
    return np.stack([res.results[b]["y"] for b in range(B)]).astype(np.float32)


# revision 9
# speedup vs baseline: 1.5820x; 1.0346x over previous
"""EquivariantAttention Trainium2 kernel, v2.

B=8 batches data-parallel over 8 NeuronCores; per core (n=2048, H=512):

  qkv = x @ W_qkv + b ; only q,k,v'' kept where v'' = x @ (W_v@W_out) + b''
  (W_out folded into V so the final projection disappears; b'' = b_v@W_out+b_out
   and the +b_out part rides the softmax identity sum_j p_ij = 1).

  E = exp(-sqrt(d2 + eps)) precomputed f16 for ALL i-chunks in the prologue
  (one Sqrt table block + one Exp table block = 2 ACT table loads total);
  d2 computed on PE from augmented geometry, sqrt reads PSUM directly
  (eps bias replaces the DVE clamp).

  scores S^T[j,i] = k8^T q8 via fp8e4 DoubleRow matmuls (K=256/instr,
  0.5 cycles/row); q split hi+lo fp8 for precision, k single fp8.
  U = exp(S^T*E/sqrt(H)) as f16, W = U-1 quantized e4m3 (near-1 precision),
  attnV natural layout: out[i,h] = colsum(v'') + W8 @ (v_hi8 + v_lo8) via
  DoubleRow; rowsums l = 2048 + W8 @ ones via ap=1 DR matmuls; y = out/l.

Engine split: PE matmuls; ACT sqrt/exp/ynorm; DVE psum copybacks + s*E;
Pool (SBUF-only operands for HW safety) dtype converts + W=U-1 subtract.
"""

import numpy as np

import concourse.bass as bass
from concourse import bacc
import concourse.mybir as mybir
import concourse.tile as tile
from concourse.masks import make_identity
from concourse.tile import add_dep_helper

P = 128
H = 512
SC = 512
HT = H // P  # 4

f32 = mybir.dt.float32
f32r = mybir.dt.float32r
bf16 = mybir.dt.bfloat16
f16 = mybir.dt.float16
fp8 = mybir.dt.float8e4
AF = mybir.ActivationFunctionType
OP = mybir.AluOpType
DR = mybir.MatmulPerfMode.DoubleRow
RSQRT_H = 1.0 / float(np.sqrt(H))
D2_EPS = 1e-4
EPS_OVERRIDE = [None]


def _body(tc, n, x, g, wqkv, bqkv, wout, bout, y, dbg=None):
    nc = tc.nc
    NT = n // P           # j-tiles (16)
    NC_ = n // SC         # i-chunks (4)
    ITC = SC // P         # i-tiles per chunk (4)
    NH = 2                # x-DMA chunk, in units of nt
    NJJ = NT // 2         # j pair-tiles (8)

    state = {"prev": None}

    def chain(a):
        # keep ACT in emission order so table-set switches stay batched
        if state["prev"] is not None:
            add_dep_helper(a.ins, state["prev"].ins, sync=False,
                           reason="ACT table-set batching")
        state["prev"] = a
        return a

    with (
        nc.allow_low_precision(
            reason="fp8/bf16 attention; fp32r transposes and d2"
        ),
        tc.tile_pool(name="const", bufs=1) as const,
        tc.tile_pool(name="attn", bufs=1) as attn,
        tc.tile_pool(name="etp", bufs=2) as etp,
        tc.tile_pool(name="u16p", bufs=2) as u16p,
        tc.tile_pool(name="w8p", bufs=2) as w8p,
        tc.tile_pool(name="yp", bufs=2) as yp,
        tc.tile_pool(name="lp", bufs=2) as lp,
    ):
        # ---------------- constants ----------------
        ident = const.tile([P, P], f32)
        make_identity(nc, ident)
        eps_ap = const.tile([P, 1], f32)
        nc.vector.memset(eps_ap, EPS_OVERRIDE[0] or D2_EPS)
        neg1_ap = const.tile([P, 1], f32)
        nc.vector.memset(neg1_ap, -1.0)
        bqk_sb = const.tile([P, 8], f32)  # cols 0-3 b_q m-tiles, 4-7 b_k
        nc.sync.dma_start(bqk_sb, bqkv[0 : 2 * H].rearrange("(mt p) -> p mt", p=P))
        ones8 = const.tile([P, 2, 1], fp8)
        nc.vector.memset(ones8, 1.0)
        ones_row = const.tile([1, P], f32r)
        nc.vector.memset(ones_row.bitcast(f32), 1.0)
        cN = const.tile([1, ITC], f32r)
        nc.vector.memset(cN.bitcast(f32), float(n))
        colsum_row = const.tile([1, H], f32r)  # written in prologue

        # ---------------- persistent attention tiles ----------------
        qhT = attn.tile([P, HT, n], fp8)   # q hi, [h, i] natural scale
        qlT = attn.tile([P, HT, n], fp8)   # q lo residual
        kT = attn.tile([P, HT, n], fp8)    # k, [h, j]
        Es = attn.tile([P, NC_, NT, SC], f16)  # exp(-dist), [j, i]
        v8h = attn.tile([P, NT, H], fp8)   # v'' hi, [j, h]
        v8l = attn.tile([P, NT, H], fp8)   # v'' lo

        # ================= prologue =================
        with (
            tc.tile_pool(name="geo", bufs=1) as geo,
            tc.tile_pool(name="wsb", bufs=1) as wsb,
            tc.tile_pool(name="xsb", bufs=4) as xsb,
            tc.tile_pool(name="xtc", bufs=2) as xtc,
            tc.tile_pool(name="vt", bufs=2) as vt,
            tc.tile_pool(name="ps_q", bufs=3, space="PSUM") as ps_q,  # 3x1
            tc.tile_pool(name="ps_d", bufs=2, space="PSUM") as ps_d,  # 2x2
            tc.tile_pool(name="ps_one", bufs=1, space="PSUM") as ps_one,  # 1
        ):
            # -- DMA priority order (single DMA engine serializes): g, x0,
            # Wv+Wout pieces (W' needed early), then x chunks alternating
            # with remaining W_qkv pieces; weights stage f32->bf16 on Pool.
            WPC = 128
            g_sb = wsb.tile([P, NT, 3], f32)
            nc.sync.dma_start(g_sb, g.rearrange("(nt p) c -> p nt c", p=P))
            x_r = x.rearrange("(nt p) h -> p nt h", p=P)
            wq_r = wqkv.rearrange("(kt p) m -> p kt m", p=P)
            wo_r = wout.rearrange("(kt p) m -> p kt m", p=P)
            wqkv_bf = wsb.tile([P, HT, 3 * H], bf16)
            wout_bf = wsb.tile([P, HT, H], bf16)

            def stage_w(dst_bf, src_ap):
                # DMA issue on the (otherwise idle) SP queue so the Pool
                # converts don't serialize the staging DMAs behind them
                ws = xsb.tile([P, HT, WPC], f32, tag="ws")
                nc.sync.dma_start(ws, src_ap)
                nc.gpsimd.tensor_copy(dst_bf, ws)

            x_stage = []

            def dma_x(i):
                xs = xsb.tile([P, NH, H], f32, tag="xs")
                nc.sync.dma_start(xs, x_r[:, i * NH : (i + 1) * NH, :])
                x_stage.append(xs)

            for i in range(min(4, NT // NH)):
                dma_x(i)
            for pc in range(8, 12):  # Wv
                stage_w(wqkv_bf[:, :, pc * WPC : (pc + 1) * WPC],
                        wq_r[:, :, pc * WPC : (pc + 1) * WPC])
            for pc in range(4):  # Wout
                stage_w(wout_bf[:, :, pc * WPC : (pc + 1) * WPC],
                        wo_r[:, :, pc * WPC : (pc + 1) * WPC])
            nxt = [0]

            def stage_wq4():
                for _ in range(4):
                    pc = nxt[0]
                    if pc < 8:
                        stage_w(wqkv_bf[:, :, pc * WPC : (pc + 1) * WPC],
                                wq_r[:, :, pc * WPC : (pc + 1) * WPC])
                        nxt[0] = pc + 1

            for i in range(4, NT // NH):
                if i in (4, 5):
                    stage_wq4()
                dma_x(i)
            while nxt[0] < 8:
                stage_wq4()
            bv_sb = wsb.tile([P, HT], f32)
            nc.gpsimd.dma_start(
                bv_sb, bqkv[2 * H : 3 * H].rearrange("(kt p) -> p kt", p=P)
            )
            bpp_row = wsb.tile([1, H], f32r)
            nc.gpsimd.dma_start(
                bpp_row,
                bout.rearrange("(one h) -> one h", one=1).bitcast(f32r))
            bv_bf = wsb.tile([P, HT], bf16)
            nc.gpsimd.tensor_copy(bv_bf, bv_sb)

            # -- augmented geometry in EXACT f16: d2 = |g16_i - g16_j|^2
            # via one K=8 f16 matmul; f16xf16 products are exact in the f32
            # accumulator so d2 >= ~-1e-5 and sqrt(d2+eps) never NaNs on HW
            # (f32r-rounded operands gave d2 errors of +-6e-3 -> NaNs).
            # lhsT rows: [-2g(3), sq_hi, sq_lo, 1, 1, 0]
            # rhs  rows: [ g(3),  1,    1,     sq_hi, sq_lo, 0]
            ident16 = wsb.tile([P, P], f16)
            nc.vector.tensor_copy(ident16, ident)
            ident_bf = wsb.tile([P, P], bf16)
            nc.vector.tensor_copy(ident_bf, ident)
            hT8 = geo.tile([8, n], f16)
            gT8 = geo.tile([8, n], f16)
            g16 = wsb.tile([P, NT, 3], f16)
            nc.vector.tensor_copy(g16, g_sb)
            g2 = wsb.tile([P, NT, 3], f32)
            nc.vector.tensor_mul(g2, g16, g16)
            sq = wsb.tile([P, NT, 1], f32)
            nc.vector.reduce_sum(sq, g2, axis=mybir.AxisListType.X)
            sqh = wsb.tile([P, NT, 1], f16)
            nc.vector.tensor_copy(sqh, sq)
            sql = wsb.tile([P, NT, 1], f16)
            nc.vector.tensor_sub(sql, sq, sqh)
            Ag = wsb.tile([P, NT, 8], f16)
            Ah = wsb.tile([P, NT, 8], f16)
            nc.vector.memset(Ag, 0.0)
            nc.vector.memset(Ah, 0.0)
            nc.vector.tensor_copy(Ag[:, :, 0:3], g16)
            nc.vector.memset(Ag[:, :, 3:5], 1.0)
            nc.vector.tensor_copy(Ag[:, :, 5:6], sqh)
            nc.vector.tensor_copy(Ag[:, :, 6:7], sql)
            nc.vector.tensor_scalar_mul(Ah[:, :, 0:3], g16, -2.0)
            nc.vector.tensor_copy(Ah[:, :, 3:4], sqh)
            nc.vector.tensor_copy(Ah[:, :, 4:5], sql)
            nc.vector.memset(Ah[:, :, 5:7], 1.0)
            for q4 in range(NT // 4):
                for srcA, dstT in ((Ah, hT8), (Ag, gT8)):
                    pq32 = ps_q.tile([P, 4, P], f32, tag="psq")
                    pq = pq32.bitcast(f16)  # [P, 4, 2P]
                    for j in range(4):
                        nt = q4 * 4 + j
                        nc.tensor.transpose(
                            pq[:8, j, 0:P], srcA[:, nt, :], ident16
                        )
                    nc.scalar.copy(
                        dstT[:, q4 * 4 * P : (q4 + 1) * 4 * P],
                        pq[:8, :, 0:P],
                    )

            def emit_d2_chunk(ic):
                # d2 for i-chunk ic: pair-tiles -> psd -> ACT sqrt -> Es f16
                isl = slice(ic * SC, (ic + 1) * SC)
                for jj in range(NJJ):
                    psd = ps_d.tile([P, 2, SC], f32, tag="psd")
                    for s2 in range(2):
                        jt = jj * 2 + s2
                        nc.tensor.matmul(
                            psd[:, s2, :],
                            lhsT=hT8[:, jt * P : (jt + 1) * P],
                            rhs=gT8[:, isl],
                            start=True,
                            stop=True,
                        )
                    chain(
                        nc.scalar.activation(
                            Es[:, ic, jj * 2 : jj * 2 + 2, :],
                            psd,
                            AF.Sqrt,
                            bias=eps_ap,
                        )
                    )

            # -- per n-chunk: x transposes -> xTc (bf16), d2, v'', q/k proj
            # (proj delayed one chunk so its W_qkv pieces have arrived)
            Wp = wsb.tile([P, HT, H], bf16)  # W' = Wv @ Wout, k on partitions
            WvT = wsb.tile([P, HT, H], bf16)  # Wv^T: [h', k]
            pcs = ps_one.tile([1, H], f32, tag="pcs")
            xtc_tiles = {}

            def emit_wprep():
                for kt in range(HT):
                    pq32 = ps_q.tile([P, HT, P], f32, tag="psq")
                    pq = pq32.bitcast(bf16)  # [P, HT, 2*P]
                    for ht in range(HT):
                        nc.tensor.transpose(
                            pq[:, ht, 0:P],
                            wqkv_bf[:, kt,
                                    2 * H + ht * P : 2 * H + (ht + 1) * P],
                            ident_bf,
                        )
                    nc.vector.tensor_copy(
                        WvT[:, :, kt * P : (kt + 1) * P], pq[:, :, 0:P]
                    )
                for kt in range(HT):
                    psp = ps_q.tile([P, 4, P], f32, tag="psq")
                    for hp in range(HT):
                        nc.tensor.matmul(
                            psp,
                            lhsT=WvT[:, hp, kt * P : (kt + 1) * P],
                            rhs=wout_bf[:, hp, :],
                            start=(hp == 0),
                            stop=(hp == HT - 1),
                        )
                    nc.vector.tensor_copy(Wp[:, kt, :], psp)
                # b'' = b_v @ W_out + b_out, broadcast to [P, H]
                psb4 = ps_q.tile([P, 4, P], f32, tag="psq")
                psb = psb4[0:1, :, :]
                for kt in range(HT):
                    nc.tensor.matmul(
                        psb,
                        lhsT=bv_bf[:, kt : kt + 1],
                        rhs=wout_bf[:, kt, :],
                        start=(kt == 0),
                        stop=(kt == HT - 1),
                    )
                nc.vector.tensor_add(bpp_row, psb, bpp_row)

            def emit_vchunk(cv):
                xTc = xtc_tiles[cv]
                for ntl in range(ITC):
                    nt = cv * ITC + ntl
                    psp = ps_q.tile([P, 4, P], f32, tag="psq")
                    nc.tensor.matmul(
                        psp,
                        lhsT=ones_row,
                        rhs=bpp_row,
                        start=True,
                        stop=False,
                        skip_group_check=True,
                    )
                    for kc in range(HT):
                        nc.tensor.matmul(
                            psp,
                            lhsT=xTc[:, kc, ntl * P : (ntl + 1) * P],
                            rhs=Wp[:, kc, :],
                            start=False,
                            stop=(kc == HT - 1),
                            skip_group_check=True,
                        )
                    vtmp = vt.tile([P, H], f32, tag="vt")
                    nc.vector.tensor_copy(vtmp, psp)
                    nc.gpsimd.tensor_copy(v8h[:, nt, :], vtmp)
                    nc.vector.tensor_sub(v8l[:, nt, :], vtmp, v8h[:, nt, :])

            def emit_qkproj(c):
                csl = slice(c * SC, (c + 1) * SC)
                xTc = xtc_tiles.pop(c)
                for mt in [0, 1, 2, 3, 4, 5, 6, 7]:
                    mi = mt % 4
                    psp = ps_q.tile([P, 4, P], f32, tag="psq")
                    for kc in range(HT):
                        nc.tensor.matmul(
                            psp,
                            lhsT=wqkv_bf[:, kc, mt * P : (mt + 1) * P],
                            rhs=xTc[:, kc, :],
                            start=(kc == 0),
                            stop=(kc == HT - 1),
                        )
                    if mt >= 4:
                        nc.vector.tensor_scalar_add(
                            kT[:, mi, csl], psp, bqk_sb[:, 4 + mi : 5 + mi]
                        )
                    else:
                        nc.vector.tensor_scalar_add(
                            qhT[:, mi, csl], psp, bqk_sb[:, mi : mi + 1]
                        )
                        nc.vector.scalar_tensor_tensor(
                            qlT[:, mi, csl],
                            psp,
                            bqk_sb[:, mi : mi + 1],
                            qhT[:, mi, csl],
                            OP.add,
                            OP.subtract,
                        )

            for c in range(NC_):
                xTc = xtc.tile([P, HT, SC], bf16, tag="xtc")
                xtc_tiles[c] = xTc
                for ht in range(HT):
                    pq = ps_q.tile([P, 4, P], f32, tag="psq")
                    for j in range(4):
                        xs = x_stage[2 * c + j // NH]
                        nc.tensor.transpose(
                            pq[:, j, :],
                            xs[:, j % NH, ht * P : (ht + 1) * P],
                            ident,
                        )
                    nc.vector.tensor_copy(xTc[:, ht, :], pq)
                emit_d2_chunk(c)
                if c == min(1, NC_ - 1):
                    emit_wprep()
                for cv in ([c - 1] if c >= 1 else ([] if NC_ > 1 else [0])):
                    emit_vchunk(cv)
                if c >= 1:
                    emit_qkproj(c - 1)
            if NC_ > 1:
                emit_vchunk(NC_ - 1)
            emit_qkproj(NC_ - 1)
            if dbg is not None:
                nc.sync.dma_start(dbg["ht8"], hT8)
                nc.sync.dma_start(dbg["gt8"], gT8)
            # column sums of v'' from the fp8 hi/lo pair (plain fp8 matmuls;
            # DoubleRow with a 1-partition output breaks walrus codegen)
            ones81 = const.tile([P, 1], fp8)
            nc.vector.memset(ones81, 1.0)
            for v8 in (v8h, v8l):
                for jt in range(NT):
                    nc.tensor.matmul(
                        pcs,
                        lhsT=ones81,
                        rhs=v8[:, jt, :],
                        start=(v8 is v8h and jt == 0),
                        stop=(v8 is v8l and jt == NT - 1),
                    )
            nc.vector.tensor_copy(colsum_row, pcs)

            # E = exp(-dist), in place on Es (f16), 4 j-tiles per op;
            # then pin the diagonal back to exactly 1 (E<=1 everywhere, and
            # sqrt(d2+eps) biased the self-distance) via max with identity
            for ic in range(NC_):
                for jq in range(NT // 4):
                    chain(
                        nc.scalar.activation(
                            Es[:, ic, jq * 4 : (jq + 1) * 4, :],
                            Es[:, ic, jq * 4 : (jq + 1) * 4, :],
                            AF.Exp,
                            scale=-1.0,
                        )
                    )
                for it in range(ITC):
                    jd = ic * ITC + it
                    dsl = Es[:, ic, jd, it * P : (it + 1) * P]
                    nc.vector.tensor_max(dsl, dsl, ident16)

        # ================= main attention loop =================
        # open order matters: ps_o/ps_l land on the prologue's psq banks
        # (drained late by the last copybacks), ps_s lands on the d2 banks
        # (drained early) so scores c0 isn't gated on the prologue tail
        main_po = tc.tile_pool(name="ps_o", bufs=2, space="PSUM")
        ps_o = main_po.__enter__()
        main_pl = tc.tile_pool(name="ps_l", bufs=1, space="PSUM")
        ps_l = main_pl.__enter__()
        main_ps = tc.tile_pool(name="ps_s", bufs=2, space="PSUM")
        ps_s = main_ps.__enter__()
        y_r = y.rearrange("(nt p) h -> p nt h", p=P)

        def emit_scores(ic, jjs=None):
            isl = slice(ic * SC, (ic + 1) * SC)
            pss_list = []
            for jj in (jjs if jjs is not None else range(NJJ)):
                pss = ps_s.tile([P, 2, SC], f32, tag="pss")
                for s2 in range(2):
                    jt = jj * 2 + s2
                    jsl = slice(jt * P, (jt + 1) * P)
                    first = True
                    for qT in (qhT, qlT):
                        for kcp in range(HT // 2):
                            nc.tensor.matmul(
                                pss[:, s2, :],
                                lhsT=kT[:, 2 * kcp : 2 * kcp + 2, jsl],
                                rhs=qT[:, 2 * kcp : 2 * kcp + 2, isl],
                                start=first,
                                stop=(qT is qlT) and kcp == HT // 2 - 1,
                                perf_mode=DR,
                            )
                            first = False
                pss_list.append(pss)
            return pss_list

        def emit_elementwise(ic, pss_list, W8, jjs):
            # s*E on DVE, U=exp on ACT, W8=U-1 on Pool
            for i, jj in enumerate(jjs):
                et = etp.tile([P, 2, SC], f16, tag="et")
                nc.vector.tensor_mul(
                    et, pss_list[i], Es[:, ic, jj * 2 : jj * 2 + 2, :]
                )
                u16 = u16p.tile([P, 2, SC], f16, tag="u16")
                chain(nc.scalar.activation(u16, et, AF.Exp, scale=RSQRT_H))
                eng = nc.vector if jj % 4 == 3 else nc.gpsimd
                eng.tensor_scalar(
                    W8[:, jj * 2 : jj * 2 + 2, :], u16, 1.0, 400.0,
                    OP.subtract, OP.min,
                )

        def emit_attnv(ic, W8):
            # rowsums l = n + sum_j W8 (ap=1 DR matmuls into psl columns)
            psl = ps_l.tile([P, ITC], f32, tag="psl")
            for jj in range(NJJ):
                for it in range(ITC):
                    # start only on the very first matmul: the PSUM zero
                    # region is the whole 2KB bank, so a second start would
                    # wipe the other columns' partial sums
                    nc.tensor.matmul(
                        psl[:, it : it + 1],
                        lhsT=W8[:, jj * 2 : jj * 2 + 2, it * P : (it + 1) * P],
                        rhs=ones8,
                        start=(jj == 0 and it == 0),
                        stop=False,
                        perf_mode=DR,
                        skip_group_check=True,
                    )
            nc.tensor.matmul(
                psl,
                lhsT=ones_row,
                rhs=cN,
                start=False,
                stop=True,
                skip_group_check=True,
            )
            linv = lp.tile([P, ITC], f32, tag="linv")
            nc.vector.reciprocal(linv, psl)
            # out[i, h] = colsum + W8 @ (v_hi + v_lo), then y = out * (1/l)
            for it in range(ITC):
                pso = ps_o.tile([P, SC], f32, tag="pso")
                nc.tensor.matmul(
                    pso,
                    lhsT=ones_row,
                    rhs=colsum_row,
                    start=True,
                    stop=False,
                    skip_group_check=True,
                )
                itsl = slice(it * P, (it + 1) * P)
                for v8 in (v8h, v8l):
                    for jj in range(NJJ):
                        nc.tensor.matmul(
                            pso,
                            lhsT=W8[:, jj * 2 : jj * 2 + 2, itsl],
                            rhs=v8[:, jj * 2 : jj * 2 + 2, :],
                            start=False,
                            stop=(v8 is v8l) and jj == NJJ - 1,
                            perf_mode=DR,
                            skip_group_check=True,
                        )
                y_t = yp.tile([P, H], f32, tag="y")
                nc.vector.tensor_scalar_mul(y_t, pso, linv[:, it : it + 1])
                nc.sync.dma_start(y_r[:, ic * ITC + it, :], y_t)

        # software pipeline: attnV(ic-1) sits between the two score halves
        # of chunk ic so PE has ready work while W8(ic) is produced
        half1 = list(range(NJJ // 2))
        half2 = list(range(NJJ // 2, NJJ))
        W8_cur = w8p.tile([P, NT, SC], fp8, tag="w8")
        pl1 = emit_scores(0, half1)
        emit_elementwise(0, pl1, W8_cur, half1)
        pl2 = emit_scores(0, half2)
        emit_elementwise(0, pl2, W8_cur, half2)
        for ic in range(NC_):
            if ic + 1 < NC_:
                W8_nxt = w8p.tile([P, NT, SC], fp8, tag="w8")
                pl1 = emit_scores(ic + 1, half1)
                emit_elementwise(ic + 1, pl1, W8_nxt, half1)
                emit_attnv(ic, W8_cur)
                pl2 = emit_scores(ic + 1, half2)
                emit_elementwise(ic + 1, pl2, W8_nxt, half2)
                W8_cur = W8_nxt
            else:
                emit_attnv(ic, W8_cur)
        if dbg is not None:
            nc.sync.dma_start(dbg["e"], Es.rearrange("p a b c -> p (a b c)"))
            nc.sync.dma_start(dbg["qh"], qhT.rearrange("p a b -> p (a b)"))
            nc.sync.dma_start(dbg["ql"], qlT.rearrange("p a b -> p (a b)"))
            nc.sync.dma_start(dbg["k"], kT.rearrange("p a b -> p (a b)"))
            nc.sync.dma_start(dbg["vh"], v8h.rearrange("p a b -> p (a b)"))
            nc.sync.dma_start(dbg["vl"], v8l.rearrange("p a b -> p (a b)"))
            nc.sync.dma_start(dbg["cs"], colsum_row.bitcast(f32))
            nc.sync.dma_start(dbg["w8"], W8_cur.rearrange("p a b -> p (a b)"))
        main_ps.__exit__(None, None, None)
        main_pl.__exit__(None, None, None)
        main_po.__exit__(None, None, None)


def build_bass(n: int = 2048, debug: bool = False) -> bass.Bass:
    nc = bacc.Bacc(None, target_bir_lowering=False)
    x = nc.dram_tensor("x", [n, H], f32, kind="ExternalInput")[:, :]
    g = nc.dram_tensor("g", [n, 3], f32, kind="ExternalInput")[:, :]
    wqkv = nc.dram_tensor("w_qkv", [H, 3 * H], f32, kind="ExternalInput")[:, :]
    bqkv = nc.dram_tensor("b_qkv", [3 * H], f32, kind="ExternalInput")[:]
    wout = nc.dram_tensor("w_out", [H, H], f32, kind="ExternalInput")[:, :]
    bout = nc.dram_tensor("b_out", [H], f32, kind="ExternalInput")[:]
    y = nc.dram_tensor("y", [n, H], f32, kind="ExternalOutput")[:, :]
    dbg = None
    if debug:
        NT = n // P
        mk = lambda nm, shape, dt: nc.dram_tensor(
            nm, shape, dt, kind="ExternalOutput")[:, :]
        dbg = {
            "e": mk("dbg_e", [P, (n // SC) * NT * SC], f16),
            "qh": mk("dbg_qh", [P, HT * n], fp8),
            "ql": mk("dbg_ql", [P, HT * n], fp8),
            "k": mk("dbg_k", [P, HT * n], fp8),
            "vh": mk("dbg_vh", [P, NT * H], fp8),
            "vl": mk("dbg_vl", [P, NT * H], fp8),
            "cs": mk("dbg_cs", [1, H], f32),
            "w8": mk("dbg_w8", [P, NT * SC], fp8),
            "ht8": nc.dram_tensor("dbg_ht8", [8, n], f16,
                                  kind="ExternalOutput")[:, :],
            "gt8": nc.dram_tensor("dbg_gt8", [8, n], f16,
                                  kind="ExternalOutput")[:, :],
        }
    with tile.TileContext(nc) as tc:
        _body(tc, n, x, g, wqkv, bqkv, wout, bout, y, dbg=dbg)
    nc.finalize()
    return nc


_CACHED = {}


def _get_nc(n: int = 2048) -> bass.Bass:
    if n not in _CACHED:
        _CACHED[n] = build_bass(n)
    return _CACHED[n]


def kernel(**inputs) -> np.ndarray:
    from concourse.bass_utils import run_bass_kernel_spmd

    x = np.ascontiguousarray(inputs["x"], dtype=np.float32)
    g = np.ascontiguousarray(inputs["geometric_features"], dtype=np.float32)
    wqkv = np.ascontiguousarray(inputs["W_qkv"], dtype=np.float32)
    bqkv = np.ascontiguousarray(inputs["b_qkv"], dtype=np.float32)
    wout = np.ascontiguousarray(inputs["W_out"], dtype=np.float32)
    bout = np.ascontiguousarray(inputs["b_out"], dtype=np.float32)

    B, n, _ = x.shape
    nc = _get_nc(n)
    core_ids = list(range(B))
    in_maps = [
        {
            "x": np.ascontiguousarray(x[b]),
            "g": np.ascontiguousarray(g[b]),
            "w_qkv": wqkv,
            "b_qkv": bqkv,
            "w_out": wout,
            "b_out": bout,
        }
        for b in range(B)
    ]
    res = run_bass_kernel_spmd(nc, in_maps, core_ids)
    return np.stack([res.results[b]["y"] for b in range(B)]).astype(np.float32)


# revision 10
# speedup vs baseline: 1.5939x; 1.0075x over previous
"""EquivariantAttention Trainium2 kernel, v2.

B=8 batches data-parallel over 8 NeuronCores; per core (n=2048, H=512):

  qkv = x @ W_qkv + b ; only q,k,v'' kept where v'' = x @ (W_v@W_out) + b''
  (W_out folded into V so the final projection disappears; b'' = b_v@W_out+b_out
   and the +b_out part rides the softmax identity sum_j p_ij = 1).

  E = exp(-sqrt(d2 + eps)) precomputed f16 for ALL i-chunks in the prologue
  (one Sqrt table block + one Exp table block = 2 ACT table loads total);
  d2 computed on PE from augmented geometry, sqrt reads PSUM directly
  (eps bias replaces the DVE clamp).

  scores S^T[j,i] = k8^T q8 via fp8e4 DoubleRow matmuls (K=256/instr,
  0.5 cycles/row); q split hi+lo fp8 for precision, k single fp8.
  U = exp(S^T*E/sqrt(H)) as f16, W = U-1 quantized e4m3 (near-1 precision),
  attnV natural layout: out[i,h] = colsum(v'') + W8 @ (v_hi8 + v_lo8) via
  DoubleRow; rowsums l = 2048 + W8 @ ones via ap=1 DR matmuls; y = out/l.

Engine split: PE matmuls; ACT sqrt/exp/ynorm; DVE psum copybacks + s*E;
Pool (SBUF-only operands for HW safety) dtype converts + W=U-1 subtract.
"""

import numpy as np

import concourse.bass as bass
from concourse import bacc
import concourse.mybir as mybir
import concourse.tile as tile
from concourse.masks import make_identity
from concourse.tile import add_dep_helper

P = 128
H = 512
SC = 512
HT = H // P  # 4

f32 = mybir.dt.float32
f32r = mybir.dt.float32r
bf16 = mybir.dt.bfloat16
f16 = mybir.dt.float16
fp8 = mybir.dt.float8e4
AF = mybir.ActivationFunctionType
OP = mybir.AluOpType
DR = mybir.MatmulPerfMode.DoubleRow
RSQRT_H = 1.0 / float(np.sqrt(H))
D2_EPS = 1e-4
EPS_OVERRIDE = [None]


def _body(tc, n, x, g, wqkv, bqkv, wout, bout, y, dbg=None):
    nc = tc.nc
    NT = n // P           # j-tiles (16)
    NC_ = n // SC         # i-chunks (4)
    ITC = SC // P         # i-tiles per chunk (4)
    NH = 2                # x-DMA chunk, in units of nt
    NJJ = NT // 2         # j pair-tiles (8)

    state = {"prev": None}

    def chain(a):
        # keep ACT in emission order so table-set switches stay batched
        if state["prev"] is not None:
            add_dep_helper(a.ins, state["prev"].ins, sync=False,
                           reason="ACT table-set batching")
        state["prev"] = a
        return a

    with (
        nc.allow_low_precision(
            reason="fp8/bf16 attention; fp32r transposes and d2"
        ),
        tc.tile_pool(name="const", bufs=1) as const,
        tc.tile_pool(name="attn", bufs=1) as attn,
        tc.tile_pool(name="etp", bufs=2) as etp,
        tc.tile_pool(name="u16p", bufs=2) as u16p,
        tc.tile_pool(name="w8p", bufs=2) as w8p,
        tc.tile_pool(name="yp", bufs=2) as yp,
        tc.tile_pool(name="lp", bufs=2) as lp,
    ):
        # ---------------- constants ----------------
        ident = const.tile([P, P], f32)
        make_identity(nc, ident)
        eps_ap = const.tile([P, 1], f32)
        nc.vector.memset(eps_ap, EPS_OVERRIDE[0] or D2_EPS)
        neg1_ap = const.tile([P, 1], f32)
        nc.vector.memset(neg1_ap, -1.0)
        bqk_sb = const.tile([P, 8], f32)  # cols 0-3 b_q m-tiles, 4-7 b_k
        nc.sync.dma_start(bqk_sb, bqkv[0 : 2 * H].rearrange("(mt p) -> p mt", p=P))
        ones8 = const.tile([P, 2, 1], fp8)
        nc.vector.memset(ones8, 1.0)
        ones_row = const.tile([1, P], f32r)
        nc.vector.memset(ones_row.bitcast(f32), 1.0)
        cN = const.tile([1, ITC], f32r)
        nc.vector.memset(cN.bitcast(f32), float(n))
        colsum_row = const.tile([1, H], f32r)  # written in prologue

        # ---------------- persistent attention tiles ----------------
        qhT = attn.tile([P, HT, n], fp8)   # q hi, [h, i] natural scale
        qlT = attn.tile([P, HT, n], fp8)   # q lo residual
        kT = attn.tile([P, HT, n], fp8)    # k, [h, j]
        Es = attn.tile([P, NC_, NT, SC], f16)  # exp(-dist), [j, i]
        v8h = attn.tile([P, NT, H], fp8)   # v'' hi, [j, h]
        v8l = attn.tile([P, NT, H], fp8)   # v'' lo

        # ================= prologue =================
        with (
            tc.tile_pool(name="geo", bufs=1) as geo,
            tc.tile_pool(name="wsb", bufs=1) as wsb,
            tc.tile_pool(name="xsb", bufs=4) as xsb,
            tc.tile_pool(name="xtc", bufs=2) as xtc,
            tc.tile_pool(name="vt", bufs=2) as vt,
            tc.tile_pool(name="ps_q", bufs=3, space="PSUM") as ps_q,  # 3x1
            tc.tile_pool(name="ps_d", bufs=2, space="PSUM") as ps_d,  # 2x2
            tc.tile_pool(name="ps_one", bufs=1, space="PSUM") as ps_one,  # 1
        ):
            # -- DMA priority order (single DMA engine serializes): g, x0,
            # Wv+Wout pieces (W' needed early), then x chunks alternating
            # with remaining W_qkv pieces; weights stage f32->bf16 on Pool.
            WPC = 128
            g_sb = wsb.tile([P, NT, 3], f32)
            nc.sync.dma_start(g_sb, g.rearrange("(nt p) c -> p nt c", p=P))
            x_r = x.rearrange("(nt p) h -> p nt h", p=P)
            wq_r = wqkv.rearrange("(kt p) m -> p kt m", p=P)
            wo_r = wout.rearrange("(kt p) m -> p kt m", p=P)
            wqkv_bf = wsb.tile([P, HT, 3 * H], bf16)
            wout_bf = wsb.tile([P, HT, H], bf16)

            def stage_w(dst_bf, src_ap):
                # DMA issue on the (otherwise idle) SP queue so the Pool
                # converts don't serialize the staging DMAs behind them
                ws = xsb.tile([P, HT, WPC], f32, tag="ws")
                nc.sync.dma_start(ws, src_ap)
                nc.gpsimd.tensor_copy(dst_bf, ws)

            x_stage = []

            def dma_x(i):
                xs = xsb.tile([P, NH, H], f32, tag="xs")
                nc.sync.dma_start(xs, x_r[:, i * NH : (i + 1) * NH, :])
                x_stage.append(xs)

            for i in range(min(4, NT // NH)):
                dma_x(i)
            for pc in range(8, 12):  # Wv
                stage_w(wqkv_bf[:, :, pc * WPC : (pc + 1) * WPC],
                        wq_r[:, :, pc * WPC : (pc + 1) * WPC])
            for pc in range(4):  # Wout
                stage_w(wout_bf[:, :, pc * WPC : (pc + 1) * WPC],
                        wo_r[:, :, pc * WPC : (pc + 1) * WPC])
            nxt = [0]

            def stage_wq4():
                for _ in range(4):
                    pc = nxt[0]
                    if pc < 8:
                        stage_w(wqkv_bf[:, :, pc * WPC : (pc + 1) * WPC],
                                wq_r[:, :, pc * WPC : (pc + 1) * WPC])
                        nxt[0] = pc + 1

            for i in range(4, NT // NH):
                if i in (4, 5):
                    stage_wq4()
                dma_x(i)
            while nxt[0] < 8:
                stage_wq4()
            bv_sb = wsb.tile([P, HT], f32)
            nc.gpsimd.dma_start(
                bv_sb, bqkv[2 * H : 3 * H].rearrange("(kt p) -> p kt", p=P)
            )
            bpp_row = wsb.tile([1, H], f32r)
            nc.gpsimd.dma_start(
                bpp_row,
                bout.rearrange("(one h) -> one h", one=1).bitcast(f32r))
            bv_bf = wsb.tile([P, HT], bf16)
            nc.gpsimd.tensor_copy(bv_bf, bv_sb)

            # -- augmented geometry in EXACT f16: d2 = |g16_i - g16_j|^2
            # via one K=8 f16 matmul; f16xf16 products are exact in the f32
            # accumulator so d2 >= ~-1e-5 and sqrt(d2+eps) never NaNs on HW
            # (f32r-rounded operands gave d2 errors of +-6e-3 -> NaNs).
            # lhsT rows: [-2g(3), sq_hi, sq_lo, 1, 1, 0]
            # rhs  rows: [ g(3),  1,    1,     sq_hi, sq_lo, 0]
            ident16 = wsb.tile([P, P], f16)
            nc.vector.tensor_copy(ident16, ident)
            ident_bf = wsb.tile([P, P], bf16)
            nc.vector.tensor_copy(ident_bf, ident)
            hT8 = geo.tile([8, n], f16)
            gT8 = geo.tile([8, n], f16)
            g16 = wsb.tile([P, NT, 3], f16)
            nc.vector.tensor_copy(g16, g_sb)
            g2 = wsb.tile([P, NT, 3], f32)
            nc.vector.tensor_mul(g2, g16, g16)
            sq = wsb.tile([P, NT, 1], f32)
            nc.vector.reduce_sum(sq, g2, axis=mybir.AxisListType.X)
            sqh = wsb.tile([P, NT, 1], f16)
            nc.vector.tensor_copy(sqh, sq)
            sql = wsb.tile([P, NT, 1], f16)
            nc.vector.tensor_sub(sql, sq, sqh)
            Ag = wsb.tile([P, NT, 8], f16)
            Ah = wsb.tile([P, NT, 8], f16)
            nc.vector.memset(Ag, 0.0)
            nc.vector.memset(Ah, 0.0)
            nc.vector.tensor_copy(Ag[:, :, 0:3], g16)
            nc.vector.memset(Ag[:, :, 3:5], 1.0)
            nc.vector.tensor_copy(Ag[:, :, 5:6], sqh)
            nc.vector.tensor_copy(Ag[:, :, 6:7], sql)
            nc.vector.tensor_scalar_mul(Ah[:, :, 0:3], g16, -2.0)
            nc.vector.tensor_copy(Ah[:, :, 3:4], sqh)
            nc.vector.tensor_copy(Ah[:, :, 4:5], sql)
            nc.vector.memset(Ah[:, :, 5:7], 1.0)
            for q4 in range(NT // 4):
                for srcA, dstT in ((Ah, hT8), (Ag, gT8)):
                    pq32 = ps_q.tile([P, 4, P], f32, tag="psq")
                    pq = pq32.bitcast(f16)  # [P, 4, 2P]
                    for j in range(4):
                        nt = q4 * 4 + j
                        nc.tensor.transpose(
                            pq[:8, j, 0:P], srcA[:, nt, :], ident16
                        )
                    nc.scalar.copy(
                        dstT[:, q4 * 4 * P : (q4 + 1) * 4 * P],
                        pq[:8, :, 0:P],
                    )

            def emit_d2_chunk(ic):
                # d2 for i-chunk ic: pair-tiles -> psd -> ACT sqrt -> Es f16
                isl = slice(ic * SC, (ic + 1) * SC)
                for jj in range(NJJ):
                    psd = ps_d.tile([P, 2, SC], f32, tag="psd")
                    for s2 in range(2):
                        jt = jj * 2 + s2
                        nc.tensor.matmul(
                            psd[:, s2, :],
                            lhsT=hT8[:, jt * P : (jt + 1) * P],
                            rhs=gT8[:, isl],
                            start=True,
                            stop=True,
                        )
                    chain(
                        nc.scalar.activation(
                            Es[:, ic, jj * 2 : jj * 2 + 2, :],
                            psd,
                            AF.Sqrt,
                            bias=eps_ap,
                        )
                    )

            # -- per n-chunk: x transposes -> xTc (bf16), d2, v'', q/k proj
            # (proj delayed one chunk so its W_qkv pieces have arrived)
            Wp = wsb.tile([P, HT, H], bf16)  # W' = Wv @ Wout, k on partitions
            WvT = wsb.tile([P, HT, H], bf16)  # Wv^T: [h', k]
            pcs = ps_one.tile([1, H], f32, tag="pcs")
            xtc_tiles = {}

            def emit_wprep():
                for kt in range(HT):
                    pq32 = ps_q.tile([P, HT, P], f32, tag="psq")
                    pq = pq32.bitcast(bf16)  # [P, HT, 2*P]
                    for ht in range(HT):
                        nc.tensor.transpose(
                            pq[:, ht, 0:P],
                            wqkv_bf[:, kt,
                                    2 * H + ht * P : 2 * H + (ht + 1) * P],
                            ident_bf,
                        )
                    nc.vector.tensor_copy(
                        WvT[:, :, kt * P : (kt + 1) * P], pq[:, :, 0:P]
                    )
                for kt in range(HT):
                    psp = ps_q.tile([P, 4, P], f32, tag="psq")
                    for hp in range(HT):
                        nc.tensor.matmul(
                            psp,
                            lhsT=WvT[:, hp, kt * P : (kt + 1) * P],
                            rhs=wout_bf[:, hp, :],
                            start=(hp == 0),
                            stop=(hp == HT - 1),
                        )
                    nc.vector.tensor_copy(Wp[:, kt, :], psp)
                # b'' = b_v @ W_out + b_out, broadcast to [P, H]
                psb4 = ps_q.tile([P, 4, P], f32, tag="psq")
                psb = psb4[0:1, :, :]
                for kt in range(HT):
                    nc.tensor.matmul(
                        psb,
                        lhsT=bv_bf[:, kt : kt + 1],
                        rhs=wout_bf[:, kt, :],
                        start=(kt == 0),
                        stop=(kt == HT - 1),
                    )
                nc.vector.tensor_add(bpp_row, psb, bpp_row)

            def emit_vchunk(cv):
                xTc = xtc_tiles[cv]
                for ntl in range(ITC):
                    nt = cv * ITC + ntl
                    psp = ps_q.tile([P, 4, P], f32, tag="psq")
                    nc.tensor.matmul(
                        psp,
                        lhsT=ones_row,
                        rhs=bpp_row,
                        start=True,
                        stop=False,
                        skip_group_check=True,
                    )
                    for kc in range(HT):
                        nc.tensor.matmul(
                            psp,
                            lhsT=xTc[:, kc, ntl * P : (ntl + 1) * P],
                            rhs=Wp[:, kc, :],
                            start=False,
                            stop=(kc == HT - 1),
                            skip_group_check=True,
                        )
                    vtmp = vt.tile([P, H], f32, tag="vt")
                    nc.vector.tensor_copy(vtmp, psp)
                    nc.gpsimd.tensor_copy(v8h[:, nt, :], vtmp)
                    nc.vector.tensor_sub(v8l[:, nt, :], vtmp, v8h[:, nt, :])

            def emit_qkproj(c):
                csl = slice(c * SC, (c + 1) * SC)
                xTc = xtc_tiles.pop(c)
                for mt in [0, 1, 2, 3, 4, 5, 6, 7]:
                    mi = mt % 4
                    psp = ps_q.tile([P, 4, P], f32, tag="psq")
                    for kc in range(HT):
                        nc.tensor.matmul(
                            psp,
                            lhsT=wqkv_bf[:, kc, mt * P : (mt + 1) * P],
                            rhs=xTc[:, kc, :],
                            start=(kc == 0),
                            stop=(kc == HT - 1),
                        )
                    if mt >= 4:
                        nc.vector.tensor_scalar_add(
                            kT[:, mi, csl], psp, bqk_sb[:, 4 + mi : 5 + mi]
                        )
                    else:
                        nc.vector.tensor_scalar_add(
                            qhT[:, mi, csl], psp, bqk_sb[:, mi : mi + 1]
                        )
                        nc.vector.scalar_tensor_tensor(
                            qlT[:, mi, csl],
                            psp,
                            bqk_sb[:, mi : mi + 1],
                            qhT[:, mi, csl],
                            OP.add,
                            OP.subtract,
                        )

            for c in range(NC_):
                xTc = xtc.tile([P, HT, SC], bf16, tag="xtc")
                xtc_tiles[c] = xTc
                emit_d2_chunk(c)
                for ht in range(HT):
                    pq = ps_q.tile([P, 4, P], f32, tag="psq")
                    for j in range(4):
                        xs = x_stage[2 * c + j // NH]
                        nc.tensor.transpose(
                            pq[:, j, :],
                            xs[:, j % NH, ht * P : (ht + 1) * P],
                            ident,
                        )
                    nc.vector.tensor_copy(xTc[:, ht, :], pq)
                if c == min(1, NC_ - 1):
                    emit_wprep()
                for cv in ([c - 1] if c >= 1 else ([] if NC_ > 1 else [0])):
                    emit_vchunk(cv)
                if c >= 1:
                    emit_qkproj(c - 1)
            if NC_ > 1:
                emit_vchunk(NC_ - 1)
            emit_qkproj(NC_ - 1)
            if dbg is not None:
                nc.sync.dma_start(dbg["ht8"], hT8)
                nc.sync.dma_start(dbg["gt8"], gT8)
            # column sums of v'' from the fp8 hi/lo pair (plain fp8 matmuls;
            # DoubleRow with a 1-partition output breaks walrus codegen)
            ones81 = const.tile([P, 1], fp8)
            nc.vector.memset(ones81, 1.0)
            for v8 in (v8h, v8l):
                for jt in range(NT):
                    nc.tensor.matmul(
                        pcs,
                        lhsT=ones81,
                        rhs=v8[:, jt, :],
                        start=(v8 is v8h and jt == 0),
                        stop=(v8 is v8l and jt == NT - 1),
                    )
            nc.vector.tensor_copy(colsum_row, pcs)

            # E = exp(-dist), in place on Es (f16), 4 j-tiles per op;
            # then pin the diagonal back to exactly 1 (E<=1 everywhere, and
            # sqrt(d2+eps) biased the self-distance) via max with identity
            for ic in range(NC_):
                for jq in range(NT // 4):
                    chain(
                        nc.scalar.activation(
                            Es[:, ic, jq * 4 : (jq + 1) * 4, :],
                            Es[:, ic, jq * 4 : (jq + 1) * 4, :],
                            AF.Exp,
                            scale=-1.0,
                        )
                    )
                for it in range(ITC):
                    jd = ic * ITC + it
                    dsl = Es[:, ic, jd, it * P : (it + 1) * P]
                    nc.vector.tensor_max(dsl, dsl, ident16)

        # ================= main attention loop =================
        # open order matters: ps_o/ps_l land on the prologue's psq banks
        # (drained late by the last copybacks), ps_s lands on the d2 banks
        # (drained early) so scores c0 isn't gated on the prologue tail
        main_po = tc.tile_pool(name="ps_o", bufs=2, space="PSUM")
        ps_o = main_po.__enter__()
        main_pl = tc.tile_pool(name="ps_l", bufs=1, space="PSUM")
        ps_l = main_pl.__enter__()
        main_ps = tc.tile_pool(name="ps_s", bufs=2, space="PSUM")
        ps_s = main_ps.__enter__()
        y_r = y.rearrange("(nt p) h -> p nt h", p=P)

        def emit_scores(ic, jjs=None):
            isl = slice(ic * SC, (ic + 1) * SC)
            pss_list = []
            for jj in (jjs if jjs is not None else range(NJJ)):
                pss = ps_s.tile([P, 2, SC], f32, tag="pss")
                for s2 in range(2):
                    jt = jj * 2 + s2
                    jsl = slice(jt * P, (jt + 1) * P)
                    first = True
                    for qT in (qhT, qlT):
                        for kcp in range(HT // 2):
                            nc.tensor.matmul(
                                pss[:, s2, :],
                                lhsT=kT[:, 2 * kcp : 2 * kcp + 2, jsl],
                                rhs=qT[:, 2 * kcp : 2 * kcp + 2, isl],
                                start=first,
                                stop=(qT is qlT) and kcp == HT // 2 - 1,
                                perf_mode=DR,
                            )
                            first = False
                pss_list.append(pss)
            return pss_list

        def emit_elementwise(ic, pss_list, W8, jjs):
            # s*E on DVE, U=exp on ACT, W8=U-1 on Pool
            for i, jj in enumerate(jjs):
                et = etp.tile([P, 2, SC], f16, tag="et")
                nc.vector.tensor_mul(
                    et, pss_list[i], Es[:, ic, jj * 2 : jj * 2 + 2, :]
                )
                u16 = u16p.tile([P, 2, SC], f16, tag="u16")
                chain(nc.scalar.activation(u16, et, AF.Exp, scale=RSQRT_H))
                eng = nc.vector if jj % 4 == 3 else nc.gpsimd
                eng.tensor_scalar(
                    W8[:, jj * 2 : jj * 2 + 2, :], u16, 1.0, 400.0,
                    OP.subtract, OP.min,
                )

        def emit_attnv(ic, W8):
            # rowsums l = n + sum_j W8 (ap=1 DR matmuls into psl columns)
            psl = ps_l.tile([P, ITC], f32, tag="psl")
            for jj in range(NJJ):
                for it in range(ITC):
                    # start only on the very first matmul: the PSUM zero
                    # region is the whole 2KB bank, so a second start would
                    # wipe the other columns' partial sums
                    nc.tensor.matmul(
                        psl[:, it : it + 1],
                        lhsT=W8[:, jj * 2 : jj * 2 + 2, it * P : (it + 1) * P],
                        rhs=ones8,
                        start=(jj == 0 and it == 0),
                        stop=False,
                        perf_mode=DR,
                        skip_group_check=True,
                    )
            nc.tensor.matmul(
                psl,
                lhsT=ones_row,
                rhs=cN,
                start=False,
                stop=True,
                skip_group_check=True,
            )
            linv = lp.tile([P, ITC], f32, tag="linv")
            nc.vector.reciprocal(linv, psl)
            # out[i, h] = colsum + W8 @ (v_hi + v_lo), then y = out * (1/l)
            for it in range(ITC):
                pso = ps_o.tile([P, SC], f32, tag="pso")
                nc.tensor.matmul(
                    pso,
                    lhsT=ones_row,
                    rhs=colsum_row,
                    start=True,
                    stop=False,
                    skip_group_check=True,
                )
                itsl = slice(it * P, (it + 1) * P)
                for v8 in (v8h, v8l):
                    for jj in range(NJJ):
                        nc.tensor.matmul(
                            pso,
                            lhsT=W8[:, jj * 2 : jj * 2 + 2, itsl],
                            rhs=v8[:, jj * 2 : jj * 2 + 2, :],
                            start=False,
                            stop=(v8 is v8l) and jj == NJJ - 1,
                            perf_mode=DR,
                            skip_group_check=True,
                        )
                y_t = yp.tile([P, H], f32, tag="y")
                nc.vector.tensor_scalar_mul(y_t, pso, linv[:, it : it + 1])
                nc.sync.dma_start(y_r[:, ic * ITC + it, :], y_t)

        # software pipeline: attnV(ic-1) sits between the two score halves
        # of chunk ic so PE has ready work while W8(ic) is produced
        half1 = list(range(NJJ // 2))
        half2 = list(range(NJJ // 2, NJJ))
        W8_cur = w8p.tile([P, NT, SC], fp8, tag="w8")
        pl1 = emit_scores(0, half1)
        emit_elementwise(0, pl1, W8_cur, half1)
        pl2 = emit_scores(0, half2)
        emit_elementwise(0, pl2, W8_cur, half2)
        for ic in range(NC_):
            if ic + 1 < NC_:
                W8_nxt = w8p.tile([P, NT, SC], fp8, tag="w8")
                pl1 = emit_scores(ic + 1, half1)
                emit_elementwise(ic + 1, pl1, W8_nxt, half1)
                emit_attnv(ic, W8_cur)
                pl2 = emit_scores(ic + 1, half2)
                emit_elementwise(ic + 1, pl2, W8_nxt, half2)
                W8_cur = W8_nxt
            else:
                emit_attnv(ic, W8_cur)
        if dbg is not None:
            nc.sync.dma_start(dbg["e"], Es.rearrange("p a b c -> p (a b c)"))
            nc.sync.dma_start(dbg["qh"], qhT.rearrange("p a b -> p (a b)"))
            nc.sync.dma_start(dbg["ql"], qlT.rearrange("p a b -> p (a b)"))
            nc.sync.dma_start(dbg["k"], kT.rearrange("p a b -> p (a b)"))
            nc.sync.dma_start(dbg["vh"], v8h.rearrange("p a b -> p (a b)"))
            nc.sync.dma_start(dbg["vl"], v8l.rearrange("p a b -> p (a b)"))
            nc.sync.dma_start(dbg["cs"], colsum_row.bitcast(f32))
            nc.sync.dma_start(dbg["w8"], W8_cur.rearrange("p a b -> p (a b)"))
        main_ps.__exit__(None, None, None)
        main_pl.__exit__(None, None, None)
        main_po.__exit__(None, None, None)


def build_bass(n: int = 2048, debug: bool = False) -> bass.Bass:
    nc = bacc.Bacc(None, target_bir_lowering=False)
    x = nc.dram_tensor("x", [n, H], f32, kind="ExternalInput")[:, :]
    g = nc.dram_tensor("g", [n, 3], f32, kind="ExternalInput")[:, :]
    wqkv = nc.dram_tensor("w_qkv", [H, 3 * H], f32, kind="ExternalInput")[:, :]
    bqkv = nc.dram_tensor("b_qkv", [3 * H], f32, kind="ExternalInput")[:]
    wout = nc.dram_tensor("w_out", [H, H], f32, kind="ExternalInput")[:, :]
    bout = nc.dram_tensor("b_out", [H], f32, kind="ExternalInput")[:]
    y = nc.dram_tensor("y", [n, H], f32, kind="ExternalOutput")[:, :]
    dbg = None
    if debug:
        NT = n // P
        mk = lambda nm, shape, dt: nc.dram_tensor(
            nm, shape, dt, kind="ExternalOutput")[:, :]
        dbg = {
            "e": mk("dbg_e", [P, (n // SC) * NT * SC], f16),
            "qh": mk("dbg_qh", [P, HT * n], fp8),
            "ql": mk("dbg_ql", [P, HT * n], fp8),
            "k": mk("dbg_k", [P, HT * n], fp8),
            "vh": mk("dbg_vh", [P, NT * H], fp8),
            "vl": mk("dbg_vl", [P, NT * H], fp8),
            "cs": mk("dbg_cs", [1, H], f32),
            "w8": mk("dbg_w8", [P, NT * SC], fp8),
            "ht8": nc.dram_tensor("dbg_ht8", [8, n], f16,
                                  kind="ExternalOutput")[:, :],
            "gt8": nc.dram_tensor("dbg_gt8", [8, n], f16,
                                  kind="ExternalOutput")[:, :],
        }
    with tile.TileContext(nc) as tc:
        _body(tc, n, x, g, wqkv, bqkv, wout, bout, y, dbg=dbg)
    nc.finalize()
    return nc


_CACHED = {}


def _get_nc(n: int = 2048) -> bass.Bass:
    if n not in _CACHED:
        _CACHED[n] = build_bass(n)
    return _CACHED[n]


def kernel(**inputs) -> np.ndarray:
    from concourse.bass_utils import run_bass_kernel_spmd

    x = np.ascontiguousarray(inputs["x"], dtype=np.float32)
    g = np.ascontiguousarray(inputs["geometric_features"], dtype=np.float32)
    wqkv = np.ascontiguousarray(inputs["W_qkv"], dtype=np.float32)
    bqkv = np.ascontiguousarray(inputs["b_qkv"], dtype=np.float32)
    wout = np.ascontiguousarray(inputs["W_out"], dtype=np.float32)
    bout = np.ascontiguousarray(inputs["b_out"], dtype=np.float32)

    B, n, _ = x.shape
    nc = _get_nc(n)
    core_ids = list(range(B))
    in_maps = [
        {
            "x": np.ascontiguousarray(x[b]),
            "g": np.ascontiguousarray(g[b]),
            "w_qkv": wqkv,
            "b_qkv": bqkv,
            "w_out": wout,
            "b_out": bout,
        }
        for b in range(B)
    ]
    res = run_bass_kernel_spmd(nc, in_maps, core_ids)
    return np.stack([res.results[b]["y"] for b in range(B)]).astype(np.float32)
